# revision 1
# baseline (speedup 1.0000x reference)
"""Trainium2 Bass kernel for nn_DWTEnhancedSTGCN (B=8, T=12, N=10000, E=160000).

Strategy (N-sharded over 8 NeuronCores):
  - Each core owns 1250 dst-nodes for ALL 8 batch elements; edges are shared
    across the batch, so each edge's 96 batch-features (8b x 12t) are gathered
    ONCE per core via dma_gather (512B rows from an xT scratch in HBM).
  - Aggregation = mean over in-edges. Linearity lets us aggregate the 12-dim
    x features instead of 64-dim hidden features; mean normalization (invdeg)
    is folded into the one-hot segment-sum matmuls on the PE:
       onehot[e, j] = (iota[j] == dstloc[e]) * invdeg[dst[e]]
    (one fused DVE tensor_scalar), then aggT_block += G_chunk.T @ onehot.
  - Dense phase in [feature, node] layout with K=128 zero-padded weight
    blocks per batch (biases ride on the all-ones row 96). LayerNorm stats
    via ones-matmul column sums, stacked across node-chunks so the row math
    is batched; activations on ACT; fusion on DVE.
Host does only: sharding/reshapes, integer index-stream building, and
parameter-only weight folding. All FP math on x runs on device.
"""
import sys
import numpy as np

sys.path.insert(0, '/opt/trn_rl_repo')

B, T, N = 8, 12, 10000
OD = 64
NCORE = 8
NPC = N // NCORE          # 1250 nodes per core
NPCP = 1280               # padded local node count (10 blocks of 128)
NBLK = NPCP // 128
EPS = 1e-5
PADROW = N                # all-zero row in xT used by padding gather idxs
NROWS = N + 16
CHUNKS = [(0, 512), (512, 512), (1024, 256)]
NPAIR = B * len(CHUNKS)   # 24
GRP = 8                   # pairs per stats/softmax group
NGRP = NPAIR // GRP
REPEAT = 1                # in-kernel repetition (timing mode)

# column layout of the packed constant tile cw [128, CW_COLS]
CL = {}
_c = 0
def _cl(name, cols):
    global _c
    CL[name] = slice(_c, _c + cols)
    _c += cols
for _b in range(B):
    _cl(f'w1_{_b}', 128)   # [A_h|A_l] at rows 12b..12b+11, biases at row 96
    _cl(f'w2_{_b}', 128)   # [B_h|B_l] + u at row 96
    _cl(f'w3_{_b}', 64)    # C_l + u_c at row 96
    _cl(f'wr_{_b}', 64)    # Ag + c_r at row 96
    _cl(f'wsx_{_b}', 3)    # col sums of (high, low, res) x-parts (+bias sums)
    _cl(f'wsab_{_b}', 2)
    _cl(f'wsac_{_b}', 2)
_cl('oneshl', 2)
_cl('ones64', 1)
_cl('gv', 1)
_cl('bv', 1)
_cl('gr', 1)
_cl('br', 1)
_cl('wad', 1)
_cl('ehl', 128)
_cl('ecol0', 128)
_cl('ecol1', 128)
_cl('onesr', 64)
_cl('ident2', 64)
_cl('eps', 1)
_cl('bad', 1)
_cl('c13', 1)
CW_COLS = -(-_c // 64) * 64

_cache = {}


# ----------------------------------------------------------------- host prep
def _prep_graph(edge_index):
    src = np.asarray(edge_index[0]).astype(np.int64).ravel()
    dst = np.asarray(edge_index[1]).astype(np.int64).ravel()
    deg = np.bincount(dst, minlength=N)
    invdeg = (1.0 / np.maximum(deg, 1)).astype(np.float32)
    order = np.argsort(dst, kind='stable')
    s_s, d_s = src[order], dst[order]
    core = d_s // NPC
    local = d_s - core * NPC
    blk = local >> 7
    dstloc = local & 127
    binid = core * NBLK + blk
    counts = np.bincount(binid, minlength=NCORE * NBLK)
    return dict(s=s_s, d=d_s, core=core, binid=binid, dstloc=dstloc,
                blk=blk, counts=counts, invdeg=invdeg)


def _build_streams(g, b_pad, nseg, nch):
    stream = NBLK * b_pad
    starts = np.zeros(NCORE * NBLK, np.int64)
    np.cumsum(g['counts'][:-1], out=starts[1:])
    rank = np.arange(len(g['s'])) - starts[g['binid']]
    pos = g['core'] * stream + g['blk'] * b_pad + rank
    src_stream = np.full(NCORE * stream, PADROW, np.int64)
    dl_stream = np.zeros(NCORE * stream, np.float32)
    iv_stream = np.zeros(NCORE * stream, np.float32)
    src_stream[pos] = g['s']
    dl_stream[pos] = g['dstloc']
    iv_stream[pos] = g['invdeg'][g['d']]
    idxs, dls, ivs = [], [], []
    for c in range(NCORE):
        st = src_stream[c * stream:(c + 1) * stream]
        stp = np.full(nch * 1024, PADROW, np.int64)
        stp[:stream] = st
        t16 = stp.reshape(nch, 64, 16).transpose(2, 0, 1).reshape(16, nch * 64)
        idxs.append(np.ascontiguousarray(np.tile(t16, (8, 1)).astype(np.int16)))
        dls.append(np.ascontiguousarray(
            dl_stream[c * stream:(c + 1) * stream].reshape(nseg, 128).T))
        ivs.append(np.ascontiguousarray(
            iv_stream[c * stream:(c + 1) * stream].reshape(nseg, 128).T))
    return idxs, dls, ivs


def _fold_weights(p):
    f = lambda k: np.asarray(p[k], np.float32)
    W_ht, b_ht, W_lt, b_lt = f('W_ht'), f('b_ht'), f('W_lt'), f('b_lt')
    Ws_h, Wn_h, b_h = f('Ws_h'), f('Wn_h'), f('b_h')
    Ws_l, Wn_l, Wc_l, b_l = f('Ws_l'), f('Wn_l'), f('Wc_l'), f('b_l')
    Whr, bhr, Wlr, blr = f('Whr'), f('bhr'), f('Wlr'), f('blr')
    Wg, bg = f('Wg'), f('bg')
    Ah = W_ht @ (Ws_h + 0.2 * Whr)
    Al = W_lt @ (Ws_l + 0.2 * Wlr)
    Ag = 2.0 * Wg
    Bh, Bl, Cl = W_ht @ Wn_h, W_lt @ Wn_l, W_lt @ Wc_l
    c_h = b_ht @ (Ws_h + 0.2 * Whr) + b_h + 0.2 * bhr
    c_l = b_lt @ (Ws_l + 0.2 * Wlr) + b_l + 0.2 * blr
    c_r = bg
    u_h, u_l, u_c = b_ht @ Wn_h, b_lt @ Wn_l, b_lt @ Wc_l
    ones = np.ones((OD,), np.float32)

    cw = np.zeros((128, CW_COLS), np.float32)
    rows_b = lambda b: slice(12 * b, 12 * b + 12)
    for b in range(B):
        cw[rows_b(b), CL[f'w1_{b}']] = np.concatenate([Ah, Al], 1)
        cw[96, CL[f'w1_{b}']] = np.concatenate([c_h, c_l])
        cw[rows_b(b), CL[f'w2_{b}']] = np.concatenate([Bh, Bl], 1)
        cw[96, CL[f'w2_{b}']] = np.concatenate([u_h, u_l])
        cw[rows_b(b), CL[f'w3_{b}']] = Cl
        cw[96, CL[f'w3_{b}']] = u_c
        cw[rows_b(b), CL[f'wr_{b}']] = Ag
        cw[96, CL[f'wr_{b}']] = c_r
        cw[rows_b(b), CL[f'wsx_{b}']] = np.stack(
            [Ah @ ones, Al @ ones, Ag @ ones], 1)
        cw[96, CL[f'wsx_{b}']] = [c_h.sum(), c_l.sum(), c_r.sum()]
        cw[rows_b(b), CL[f'wsab_{b}']] = np.stack([Bh @ ones, Bl @ ones], 1)
        cw[96, CL[f'wsab_{b}']] = [u_h.sum(), u_l.sum()]
        cw[rows_b(b), CL[f'wsac_{b}']] = np.stack(
            [np.zeros(T, np.float32), Cl @ ones], 1)
        cw[96, CL[f'wsac_{b}']] = [0.0, u_c.sum()]
    oneshl = np.zeros((128, 2), np.float32)
    oneshl[:64, 0] = 1.0
    oneshl[64:, 1] = 1.0
    cw[:, CL['oneshl']] = oneshl
    cw[0:64, CL['ones64']] = 1.0
    cw[:, CL['gv']] = np.concatenate([f('g_hn'), f('g_ln')])[:, None]
    cw[:, CL['bv']] = np.concatenate([f('b_hn'), f('b_ln')])[:, None]
    cw[0:64, CL['gr']] = (0.1 * f('g_gn'))[:, None]
    cw[0:64, CL['br']] = (0.1 * f('b_gn'))[:, None]
    cw[:, CL['wad']] = (f('Wa')[:, 0] - f('Wa')[:, 1])[:, None]
    cw[0, CL['ehl']] = np.r_[np.ones(64, np.float32), np.zeros(64, np.float32)]
    cw[1, CL['ehl']] = np.r_[np.zeros(64, np.float32), np.ones(64, np.float32)]
    cw[0, CL['ecol0']] = np.r_[np.ones(64, np.float32),
                               np.zeros(64, np.float32)]
    cw[0, CL['ecol1']] = np.r_[np.zeros(64, np.float32),
                               np.ones(64, np.float32)]
    cw[0, CL['onesr']] = 1.0
    cw[:, CL['ident2']] = np.vstack([np.eye(64, dtype=np.float32),
                                     np.eye(64, dtype=np.float32)])
    ba = f('ba')
    cw[0:24, CL['eps']] = EPS
    cw[0:GRP, CL['bad']] = float(ba[0] - ba[1])
    cw[0:GRP, CL['c13']] = 1.3
    return cw, float(ba[0] - ba[1])


# -------------------------------------------------------------- bass program
def _build_program(nseg, nch, ba_diff, repeat=1):
    import concourse.tile as tile
    from concourse import bacc, mybir

    f32 = mybir.dt.float32
    i16 = mybir.dt.int16
    AF = mybir.ActivationFunctionType
    OP = mybir.AluOpType
    SEG_PER_BLK = nseg // NBLK

    nc = bacc.Bacc("TRN2", target_bir_lowering=False, debug=False,
                   enable_asserts=False, num_devices=NCORE)

    xT_d = nc.dram_tensor("xT", [NROWS, 128], f32, kind="ExternalInput")
    rep_d = nc.dram_tensor("rep_tag", [1, 64 * repeat], f32,
                           kind="ExternalInput")
    xsb_d = nc.dram_tensor("xsb", [128, NPCP], f32, kind="ExternalInput")
    cw_d = nc.dram_tensor("cw", [128, CW_COLS], f32, kind="ExternalInput")
    idx_d, dl_d, iv_d = {}, {}, {}
    for g in "bc":
        idx_d[g] = nc.dram_tensor(f"idx_{g}", [128, nch * 64], i16,
                                  kind="ExternalInput")
        dl_d[g] = nc.dram_tensor(f"dl_{g}", [128, nseg], f32,
                                 kind="ExternalInput")
        iv_d[g] = nc.dram_tensor(f"iv_{g}", [128, nseg], f32,
                                 kind="ExternalInput")
    fo_d = nc.dram_tensor("fo", [B, OD, NPCP], f32, kind="ExternalOutput")
    ho_d = nc.dram_tensor("ho", [B, OD, NPCP], f32, kind="ExternalOutput")
    lo_d = nc.dram_tensor("lo", [B, OD, NPCP], f32, kind="ExternalOutput")

    def mmg(mms):
        """Emit matmuls as one PSUM accumulation group.
        mms: list of (out_ap, lhsT_ap, rhs_ap, tile_position)."""
        nmm = len(mms)
        for i, (out, lhsT, rhs, tp) in enumerate(mms):
            nc.tensor.matmul(out, lhsT, rhs, start=(i == 0),
                             stop=(i == nmm - 1), skip_group_check=True,
                             tile_position=tp)

    with tile.TileContext(nc) as tc:
        with (
            tc.tile_pool(name="const", bufs=1) as cpool,
        ):
            cw = cpool.tile([128, CW_COLS], f32, tag="cw")
            nc.sync.dma_start(cw[:], cw_d.ap())

            def Wf(name, rows=128):
                return cw[0:rows, CL[name]]

            rep_t = cpool.tile([1, 64], f32, tag="rep")
            nc.sync.dma_start(rep_t[:], rep_d.ap()[:, 0:64])
            xsb = cpool.tile([128, NPCP], f32, tag="xsb")
            nc.sync.dma_start(xsb[:], xsb_d.ap())
            iota_t = cpool.tile([128, 128], f32, tag="iota")
            nc.gpsimd.iota(iota_t[:], pattern=[[1, 128]], base=0,
                           channel_multiplier=0,
                           allow_small_or_imprecise_dtypes=True)
            idx_t, dl_t, iv_t, aggT = {}, {}, {}, {}
            for g in "bc":
                idx_t[g] = cpool.tile([128, nch * 64], i16, tag=f"idx{g}",
                                      name=f"idx{g}")
                nc.sync.dma_start(idx_t[g][:], idx_d[g].ap())
                dl_t[g] = cpool.tile([128, nseg], f32, tag=f"dl{g}",
                                     name=f"dlt{g}")
                nc.sync.dma_start(dl_t[g][:], dl_d[g].ap())
                iv_t[g] = cpool.tile([128, nseg], f32, tag=f"iv{g}",
                                     name=f"ivt{g}")
                nc.sync.dma_start(iv_t[g][:], iv_d[g].ap())
                aggT[g] = cpool.tile([128, NPCP], f32, tag=f"agg{g}",
                                     name=f"aggT{g}")
                nc.gpsimd.memset(aggT[g][:], 0.0)

            # ---- gather + one-hot segment-sum (per graph) ----
            for _rep in range(repeat):
              with (
                  tc.tile_pool(name="gat", bufs=4) as gpool,
                  tc.tile_pool(name="oh", bufs=6) as ohpool,
                  tc.tile_pool(name="aggps", bufs=2, space="PSUM") as aggps,
              ):
                for g in "bc":
                  ps_blk = None
                  for k in range(nch):
                      gt = gpool.tile([128, 8 * 128], f32, tag="g")
                      gt3 = gt[:].rearrange("p (c e) -> p c e", e=128)
                      nc.gpsimd.dma_gather(
                          gt3, xT_d.ap(),
                          idx_t[g][:, k * 64:(k + 1) * 64],
                          num_idxs=1024, num_idxs_reg=1024, elem_size=128)
                      for c in range(8):
                          s = k * 8 + c
                          if s >= nseg:
                              break
                          r = s % SEG_PER_BLK
                          j = s // SEG_PER_BLK
                          if r == 0:
                              ps_blk = aggps.tile([128, 128], f32, tag="agg")
                          oh = ohpool.tile([128, 128], f32, tag="oh")
                          nc.vector.tensor_scalar(
                              oh[:], iota_t[:],
                              dl_t[g][:, s:s + 1], iv_t[g][:, s:s + 1],
                              OP.is_equal, OP.mult)
                          nc.tensor.matmul(
                              ps_blk[0:97, :], gt3[:, c, 0:97], oh[:],
                              start=(r == 0), stop=(r == SEG_PER_BLK - 1),
                              skip_group_check=True, tile_position=(0, 0))
                          if r == SEG_PER_BLK - 1:
                              nc.scalar.activation(
                                  aggT[g][0:97, j * 128:(j + 1) * 128],
                                  ps_blk[0:97, :], AF.Copy)

              # ---- dense phase in groups of GRP pairs ----
              pairs = [(b, c0, kl) for b in range(B) for (c0, kl) in CHUNKS]
              with (
                  tc.tile_pool(name="mainps", bufs=2, space="PSUM") as mainps,
                  tc.tile_pool(name="statps", bufs=2, space="PSUM") as statps,
                  tc.tile_pool(name="ebc", bufs=2, space="PSUM") as ebcps,
                  tc.tile_pool(name="shl", bufs=GRP + 2) as shlpool,
                  tc.tile_pool(name="sr", bufs=GRP + 2) as srpool,
                  tc.tile_pool(name="hla", bufs=GRP + 2) as hlapool,
                  tc.tile_pool(name="sq", bufs=2) as sqpool,
                  tc.tile_pool(name="ssb", bufs=2) as ssbpool,
                  tc.tile_pool(name="stg", bufs=2) as stgpool,
                  tc.tile_pool(name="tmp", bufs=2) as tmppool,
                  tc.tile_pool(name="stat", bufs=1) as statpool,
                  tc.tile_pool(name="smax", bufs=1) as smaxpool,
              ):
               for grp in range(NGRP):
                  gpairs = list(enumerate(pairs[grp * GRP:(grp + 1) * GRP]))
                  st1 = statpool.tile([3 * GRP, 512], f32, tag="st1")
                  st2 = statpool.tile([3 * GRP, 512], f32, tag="st2")
                  sdt = smaxpool.tile([GRP, 512], f32, tag="sdt")
                  nc.gpsimd.memset(st1[:], 0.0)
                  nc.gpsimd.memset(st2[:], 1.0)
                  nc.gpsimd.memset(sdt[:], 0.0)
                  shl_t, sr_t, hla_t = {}, {}, {}

                  for q, (b, c0, kl) in gpairs:
                      xr = xsb[:, c0:c0 + kl]
                      ab = aggT['b'][:, c0:c0 + kl]
                      ac = aggT['c'][:, c0:c0 + kl]

                      phl = mainps.tile([128, 512], f32, tag="phl")
                      mmg([(phl[:, 0:kl], Wf(f'w1_{b}'), xr, (0, 0)),
                           (phl[:, 0:kl], Wf(f'w2_{b}'), ab, (0, 0)),
                           (phl[64:128, 0:kl], Wf(f'w3_{b}'), ac, (0, 64))])
                      pres = mainps.tile([64, 512], f32, tag="pres")
                      mmg([(pres[:, 0:kl], Wf(f'wr_{b}'), xr, (0, 0))])

                      sh = shlpool.tile([128, 512], f32, tag="shl")
                      shl_t[q] = sh
                      nc.scalar.activation(sh[:, 0:kl], phl[:, 0:kl], AF.Copy)
                      sr = srpool.tile([64, 512], f32, tag="sr",
                                       name=f"sr{q}")
                      sr_t[q] = sr
                      nc.scalar.activation(sr[0:64, 0:kl],
                                           pres[:, 0:kl], AF.Copy)
                      sq = sqpool.tile([128, 512], f32, tag="sq")
                      nc.scalar.activation(sq[:, 0:kl], sh[:, 0:kl], AF.Square)
                      sqr = sqpool.tile([64, 512], f32, tag="sqr")
                      nc.scalar.activation(sqr[:, 0:kl],
                                           sr[0:64, 0:kl], AF.Square)

                      # stats psum: sums@0:3, sumsq_hl@32:34, sumsq_r@64:65
                      S = statps.tile([65, 512], f32, tag="S")
                      mmg([(S[0:3, 0:kl], Wf(f'wsx_{b}'), xr, (0, 0)),
                           (S[0:2, 0:kl], Wf(f'wsab_{b}'), ab, (0, 0)),
                           (S[0:2, 0:kl], Wf(f'wsac_{b}'), ac, (0, 0))])
                      mmg([(S[32:34, 0:kl], Wf('oneshl'), sq[:, 0:kl],
                            (0, 32))])
                      mmg([(S[64:65, 0:kl], Wf('ones64', 64), sqr[:, 0:kl],
                            (0, 64))])
                      ssb = ssbpool.tile([65, 512], f32, tag="ssb")
                      nc.scalar.activation(ssb[:, 0:kl], S[:, 0:kl], AF.Copy)
                      nc.sync.dma_start(st1[3 * q:3 * q + 3, 0:kl],
                                        ssb[0:3, 0:kl])
                      nc.sync.dma_start(st2[3 * q:3 * q + 2, 0:kl],
                                        ssb[32:34, 0:kl])
                      nc.sync.dma_start(st2[3 * q + 2:3 * q + 3, 0:kl],
                                        ssb[64:65, 0:kl])

                  # ---- batched stats math ----
                  m_t = statpool.tile([3 * GRP, 512], f32, tag="m")
                  nc.vector.tensor_scalar_mul(m_t[:], st1[:], 1.0 / OD)
                  q_t = statpool.tile([3 * GRP, 512], f32, tag="q")
                  nc.vector.tensor_scalar_mul(q_t[:], st2[:], 1.0 / OD)
                  msq = statpool.tile([3 * GRP, 512], f32, tag="msq")
                  nc.vector.tensor_mul(msq[:], m_t[:], m_t[:])
                  var = statpool.tile([3 * GRP, 512], f32, tag="var")
                  nc.vector.tensor_sub(var[:], q_t[:], msq[:])
                  std = statpool.tile([3 * GRP, 512], f32, tag="std")
                  nc.scalar.activation(std[:], var[:], AF.Sqrt,
                                       bias=Wf('eps', 3 * GRP))
                  rstd = statpool.tile([3 * GRP, 512], f32, tag="rstd")
                  nc.vector.reciprocal(rstd[:], std[:])
                  mrstd = statpool.tile([3 * GRP, 512], f32, tag="mrstd")
                  nc.vector.tensor_mul(mrstd[:], m_t[:], rstd[:])

                  # ---- per-pair LN apply + activations + logit diff ----
                  for q, (b, c0, kl) in gpairs:
                      sh = shl_t[q]
                      rstg = stgpool.tile([2, 512], f32, tag="rstg")
                      nc.sync.dma_start(rstg[:, 0:kl],
                                        rstd[3 * q:3 * q + 2, 0:kl])
                      mstg = stgpool.tile([2, 512], f32, tag="mstg")
                      nc.sync.dma_start(mstg[:, 0:kl],
                                        mrstd[3 * q:3 * q + 2, 0:kl])
                      rbc = ebcps.tile([128, 512], f32, tag="ebc")
                      nc.tensor.matmul(rbc[:, 0:kl], Wf('ehl', 2),
                                       rstg[:, 0:kl],
                                       start=True, stop=True,
                                       skip_group_check=True,
                                       tile_position=(0, 0))
                      mbc = ebcps.tile([128, 512], f32, tag="ebc")
                      nc.tensor.matmul(mbc[:, 0:kl], Wf('ehl', 2),
                                       mstg[:, 0:kl],
                                       start=True, stop=True,
                                       skip_group_check=True,
                                       tile_position=(0, 0))
                      t1 = tmppool.tile([128, 512], f32, tag="t1")
                      nc.vector.tensor_mul(t1[:, 0:kl], sh[:, 0:kl],
                                           rbc[:, 0:kl])
                      t2 = tmppool.tile([128, 512], f32, tag="t2")
                      nc.vector.tensor_sub(t2[:, 0:kl], t1[:, 0:kl],
                                           mbc[:, 0:kl])
                      hla = hlapool.tile([128, 512], f32, tag="hla")
                      hla_t[q] = hla
                      yh = tmppool.tile([64, 512], f32, tag="yh")
                      nc.scalar.activation(yh[:, 0:kl], t2[0:64, 0:kl],
                                           AF.Identity, bias=Wf('bv')[0:64, :],
                                           scale=Wf('gv')[0:64, :])
                      nc.vector.scalar_tensor_tensor(
                          hla[0:64, 0:kl], yh[:, 0:kl], 0.1, yh[:, 0:kl],
                          OP.mult, OP.max)
                      nc.scalar.activation(hla[64:128, 0:kl], t2[64:128, 0:kl],
                                           AF.Gelu, bias=Wf('bv')[64:128, :],
                                           scale=Wf('gv')[64:128, :])
                      nc.sync.dma_start(ho_d.ap()[b, :, c0:c0 + kl],
                                        hla[0:64, 0:kl])
                      nc.sync.dma_start(lo_d.ap()[b, :, c0:c0 + kl],
                                        hla[64:128, 0:kl])
                      sd = statps.tile([1, 512], f32, tag="S")
                      nc.tensor.matmul(sd[:, 0:kl], Wf('wad'), hla[:, 0:kl],
                                       start=True, stop=True,
                                       skip_group_check=True,
                                       tile_position=(0, 0))
                      sdb = ssbpool.tile([1, 512], f32, tag="sdb")
                      nc.scalar.activation(sdb[:, 0:kl], sd[:, 0:kl], AF.Copy)
                      nc.sync.dma_start(sdt[q:q + 1, 0:kl], sdb[:, 0:kl])

                  # ---- batched 2-way softmax ----
                  a0 = smaxpool.tile([GRP, 512], f32, tag="a0")
                  nc.scalar.activation(a0[:], sdt[:], AF.Sigmoid,
                                       bias=Wf('bad', GRP))
                  w0 = smaxpool.tile([GRP, 512], f32, tag="w0")
                  nc.vector.tensor_scalar_add(w0[:], a0[:], 0.3)
                  w1_ = smaxpool.tile([GRP, 512], f32, tag="w1_")
                  nc.scalar.activation(w1_[:], a0[:], AF.Identity,
                                       bias=Wf('c13', GRP), scale=-1.0)

                  # ---- per-pair fusion + residual + output ----
                  for q, (b, c0, kl) in gpairs:
                      hla = hla_t[q]
                      sr = sr_t[q]
                      w0s = stgpool.tile([1, 512], f32, tag="w0s")
                      nc.sync.dma_start(w0s[:, 0:kl], w0[q:q + 1, 0:kl])
                      w1s = stgpool.tile([1, 512], f32, tag="w1s")
                      nc.sync.dma_start(w1s[:, 0:kl], w1_[q:q + 1, 0:kl])
                      wbc = ebcps.tile([128, 512], f32, tag="ebc")
                      nc.tensor.matmul(wbc[:, 0:kl], Wf('ecol0', 1),
                                       w0s[:, 0:kl], start=True,
                                       stop=False, skip_group_check=True,
                                       tile_position=(0, 0))
                      nc.tensor.matmul(wbc[:, 0:kl], Wf('ecol1', 1),
                                       w1s[:, 0:kl], start=False,
                                       stop=True, skip_group_check=True,
                                       tile_position=(0, 0))
                      f1 = tmppool.tile([128, 512], f32, tag="f1")
                      nc.vector.tensor_mul(f1[:, 0:kl], hla[:, 0:kl],
                                           wbc[:, 0:kl])
                      rrs = stgpool.tile([1, 512], f32, tag="rrs")
                      nc.sync.dma_start(rrs[:, 0:kl],
                                        rstd[3 * q + 2:3 * q + 3, 0:kl])
                      rms = stgpool.tile([1, 512], f32, tag="rms")
                      nc.sync.dma_start(rms[:, 0:kl],
                                        mrstd[3 * q + 2:3 * q + 3, 0:kl])
                      rr = ebcps.tile([64, 512], f32, tag="ebc")
                      nc.tensor.matmul(rr[:, 0:kl], Wf('onesr', 1),
                                       rrs[:, 0:kl],
                                       start=True, stop=True,
                                       skip_group_check=True,
                                       tile_position=(0, 0))
                      rm = ebcps.tile([64, 512], f32, tag="ebc")
                      nc.tensor.matmul(rm[:, 0:kl], Wf('onesr', 1),
                                       rms[:, 0:kl],
                                       start=True, stop=True,
                                       skip_group_check=True,
                                       tile_position=(0, 0))
                      u1 = tmppool.tile([64, 512], f32, tag="u1")
                      nc.vector.tensor_mul(u1[:, 0:kl], sr[0:64, 0:kl],
                                           rr[:, 0:kl])
                      u2 = tmppool.tile([64, 512], f32, tag="u2")
                      nc.vector.tensor_sub(u2[:, 0:kl], u1[:, 0:kl],
                                           rm[:, 0:kl])
                      resa = tmppool.tile([64, 512], f32, tag="resa")
                      nc.scalar.activation(resa[:, 0:kl], u2[:, 0:kl],
                                           AF.Identity, bias=Wf('br')[0:64, :],
                                           scale=Wf('gr')[0:64, :])
                      f2 = ebcps.tile([64, 512], f32, tag="ebc")
                      nc.tensor.matmul(f2[:, 0:kl], Wf('ident2'), f1[:, 0:kl],
                                       start=True, stop=True,
                                       skip_group_check=True,
                                       tile_position=(0, 0))
                      f3 = tmppool.tile([64, 512], f32, tag="f3")
                      nc.vector.tensor_add(f3[:, 0:kl], f2[:, 0:kl],
                                           resa[:, 0:kl])
                      nc.sync.dma_start(fo_d.ap()[b, :, c0:c0 + kl],
                                        f3[:, 0:kl])
    nc.finalize()
    return nc


# ------------------------------------------------------------------- runner
class _SpmdRunner:
    def __init__(self, nc, n_cores=NCORE):
        import jax
        from jax.sharding import Mesh, PartitionSpec
        from jax.experimental.shard_map import shard_map
        from concourse import mybir
        from concourse.bass2jax import (_bass_exec_p, install_neuronx_cc_hook,
                                        partition_id_tensor)
        install_neuronx_cc_hook()
        self.jax = jax
        self.n_cores = n_cores
        partition_name = (nc.partition_id_tensor.name
                          if nc.partition_id_tensor else None)
        in_names, out_names, out_avals, zero_outs = [], [], [], []
        for alloc in nc.m.functions[0].allocations:
            if not isinstance(alloc, mybir.MemoryLocationSet):
                continue
            name = alloc.memorylocations[0].name
            if alloc.kind == "ExternalInput":
                if name != partition_name:
                    in_names.append(name)
            elif alloc.kind == "ExternalOutput":
                out_names.append(name)
                shape = tuple(alloc.tensor_shape)
                dtype = mybir.dt.np(alloc.dtype)
                out_avals.append(jax.core.ShapedArray(shape, dtype))
                zero_outs.append(np.zeros(shape, dtype))
        self.in_names, self.out_names = in_names, out_names
        self.out_avals = out_avals
        n_params, n_outs = len(in_names), len(out_avals)
        all_in = list(in_names) + list(out_names)
        if partition_name is not None:
            all_in.append(partition_name)

        def _body(*args):
            operands = list(args)
            if partition_name is not None:
                operands.append(partition_id_tensor())
            outs = _bass_exec_p.bind(
                *operands, out_avals=tuple(out_avals),
                in_names=tuple(all_in), out_names=tuple(out_names),
                lowering_input_output_aliases=(),
                sim_require_finite=True, sim_require_nnan=True, nc=nc)
            return tuple(outs)

        devices = jax.devices()[:n_cores]
        mesh = Mesh(np.asarray(devices), ("core",))
        in_specs = (PartitionSpec("core"),) * (n_params + n_outs)
        out_specs = (PartitionSpec("core"),) * n_outs
        self.fn = jax.jit(
            shard_map(_body, mesh=mesh, in_specs=in_specs,
                      out_specs=out_specs, check_rep=False),
            keep_unused=True)
        self._concat_zeros = [
            np.zeros((n_cores * z.shape[0], *z.shape[1:]), z.dtype)
            for z in zero_outs]

    def prepare(self, in_maps):
        n = self.n_cores
        per_core = [[np.ascontiguousarray(m[name]) for name in self.in_names]
                    for m in in_maps]
        concat_in = [np.concatenate([per_core[c][i] for c in range(n)], axis=0)
                     for i in range(len(self.in_names))]
        args = concat_in + self._concat_zeros
        return [self.jax.device_put(a) for a in args]

    def run(self, args):
        outs = self.fn(*args)
        self.jax.block_until_ready(outs)
        return outs

    def split_outs(self, outs):
        res = []
        for c in range(self.n_cores):
            d = {}
            for i, name in enumerate(self.out_names):
                d[name] = np.asarray(outs[i]).reshape(
                    self.n_cores, *self.out_avals[i].shape)[c]
            res.append(d)
        return res


# -------------------------------------------------------------------- entry
def _get(inputs):
    gb = _prep_graph(inputs['edge_index'])
    gc = _prep_graph(inputs['causal_edge_index'])
    b_pad = max(128, -(-int(max(gb['counts'].max(), gc['counts'].max()))
                     // 128) * 128)
    stream = NBLK * b_pad
    nseg = stream // 128
    nch = -(-stream // 1024)
    cw, ba_diff = _fold_weights(inputs)
    key = (b_pad, nseg, nch, round(ba_diff, 9), REPEAT)
    if key not in _cache:
        nc = _build_program(nseg, nch, ba_diff, REPEAT)
        _cache[key] = _SpmdRunner(nc)
    return _cache[key], gb, gc, b_pad, nseg, nch, cw


def make_in_maps(inputs):
    runner, gb, gc, b_pad, nseg, nch, cw = _get(inputs)
    x = np.asarray(inputs['x'], np.float32)
    xflat = np.zeros((128, N), np.float32)
    xflat[0:96] = x.reshape(96, N)
    xflat[96] = 1.0
    xT = np.zeros((NROWS, 128), np.float32)
    xT[:N, 0:97] = xflat[0:97].T
    idx_b, dl_b, iv_b = _build_streams(gb, b_pad, nseg, nch)
    idx_c, dl_c, iv_c = _build_streams(gc, b_pad, nseg, nch)
    in_maps = []
    for c in range(NCORE):
        xs = np.zeros((128, NPCP), np.float32)
        xs[:, 0:NPC] = xflat[:, c * NPC:(c + 1) * NPC]
        in_maps.append({
            'xT': xT, 'xsb': xs, 'cw': cw,
            'rep_tag': np.zeros((1, 64 * REPEAT), np.float32),
            'idx_b': idx_b[c], 'dl_b': dl_b[c], 'iv_b': iv_b[c],
            'idx_c': idx_c[c], 'dl_c': dl_c[c], 'iv_c': iv_c[c],
        })
    return runner, in_maps


def kernel(**inputs):
    runner, in_maps = make_in_maps(inputs)
    args = runner.prepare(in_maps)
    outs = runner.run(args)
    res = runner.split_outs(outs)
    fused = np.empty((B, OD, N), np.float32)
    high = np.empty((B, OD, N), np.float32)
    low = np.empty((B, OD, N), np.float32)
    for c in range(NCORE):
        sl = slice(c * NPC, (c + 1) * NPC)
        fused[:, :, sl] = res[c]['fo'][:, :, 0:NPC]
        high[:, :, sl] = res[c]['ho'][:, :, 0:NPC]
        low[:, :, sl] = res[c]['lo'][:, :, 0:NPC]
    return fused, high, low



# revision 3
# speedup vs baseline: 126.0218x; 126.0218x over previous
"""Trainium2 Bass kernel for nn_DWTEnhancedSTGCN (B=8, T=12, N=10000, E=160000).

Strategy (N-sharded over 8 NeuronCores):
  - Each core owns 1250 dst-nodes for ALL 8 batch elements; edges are shared
    across the batch, so each edge's 96 batch-features (8b x 12t) are gathered
    ONCE per core via dma_gather (512B rows from an xT scratch in HBM).
  - Aggregation = mean over in-edges. Linearity lets us aggregate the 12-dim
    x features instead of 64-dim hidden features; mean normalization (invdeg)
    is folded into the one-hot segment-sum matmuls on the PE:
       onehot[e, j] = (iota[j] == dstloc[e]) * invdeg[dst[e]]
    (one fused DVE tensor_scalar), then aggT_block += G_chunk.T @ onehot.
  - Dense phase in [feature, node] layout with K=128 zero-padded weight
    blocks per batch (biases ride on the all-ones row 96). LayerNorm stats
    via ones-matmul column sums, stacked across node-chunks so the row math
    is batched; activations on ACT; fusion on DVE.
Host does only: sharding/reshapes, integer index-stream building, and
parameter-only weight folding. All FP math on x runs on device.
"""
import sys
import numpy as np

sys.path.insert(0, '/opt/trn_rl_repo')

B, T, N = 8, 12, 10000
OD = 64
NCORE = 8
NPC = N // NCORE          # 1250 nodes per core
NPCP = 1280               # padded local node count (10 blocks of 128)
NBLK = NPCP // 128
EPS = 1e-5
PADROW = N                # all-zero row in xT used by padding gather idxs
NROWS = N + 16
CHUNKS = [(0, 512), (512, 512), (1024, 256)]
NPAIR = B * len(CHUNKS)   # 24
GRP = 8                   # pairs per stats/softmax group
NGRP = NPAIR // GRP
REPEAT = 1                # in-kernel repetition (timing mode)

# column layout of the packed constant tile cw [128, CW_COLS]
CL = {}
_c = 0
def _cl(name, cols):
    global _c
    CL[name] = slice(_c, _c + cols)
    _c += cols
for _b in range(B):
    _cl(f'w1_{_b}', 128)   # [A_h|A_l] at rows 12b..12b+11, biases at row 96
    _cl(f'w2_{_b}', 128)   # [B_h|B_l] + u at row 96
    _cl(f'w3_{_b}', 64)    # C_l + u_c at row 96
    _cl(f'wr_{_b}', 64)    # Ag + c_r at row 96
    _cl(f'wsx_{_b}', 3)    # col sums of (high, low, res) x-parts (+bias sums)
    _cl(f'wsab_{_b}', 2)
    _cl(f'wsac_{_b}', 2)
_cl('oneshl', 2)
_cl('ones64', 1)
_cl('gv', 1)
_cl('bv', 1)
_cl('gr', 1)
_cl('br', 1)
_cl('wad', 1)
_cl('ehl', 128)
_cl('ecol0', 128)
_cl('ecol1', 128)
_cl('onesr', 64)
_cl('ident2', 64)
_cl('eps', 1)
_cl('bad', 1)
_cl('c13', 1)
CW_COLS = -(-_c // 64) * 64

_cache = {}


# ----------------------------------------------------------------- host prep
def _prep_graph(edge_index):
    src = np.asarray(edge_index[0]).astype(np.int64).ravel()
    dst = np.asarray(edge_index[1]).astype(np.int64).ravel()
    deg = np.bincount(dst, minlength=N)
    invdeg = (1.0 / np.maximum(deg, 1)).astype(np.float32)
    order = np.argsort(dst, kind='stable')
    s_s, d_s = src[order], dst[order]
    core = d_s // NPC
    local = d_s - core * NPC
    blk = local >> 7
    dstloc = local & 127
    binid = core * NBLK + blk
    counts = np.bincount(binid, minlength=NCORE * NBLK)
    return dict(s=s_s, d=d_s, core=core, binid=binid, dstloc=dstloc,
                blk=blk, counts=counts, invdeg=invdeg)


def _build_streams(g, b_pad, nseg, nch):
    stream = NBLK * b_pad
    starts = np.zeros(NCORE * NBLK, np.int64)
    np.cumsum(g['counts'][:-1], out=starts[1:])
    rank = np.arange(len(g['s'])) - starts[g['binid']]
    pos = g['core'] * stream + g['blk'] * b_pad + rank
    src_stream = np.full(NCORE * stream, PADROW, np.int64)
    dl_stream = np.zeros(NCORE * stream, np.float32)
    iv_stream = np.zeros(NCORE * stream, np.float32)
    src_stream[pos] = g['s']
    dl_stream[pos] = g['dstloc']
    iv_stream[pos] = g['invdeg'][g['d']]
    idxs, dls, ivs = [], [], []
    for c in range(NCORE):
        st = src_stream[c * stream:(c + 1) * stream]
        stp = np.full(nch * 1024, PADROW, np.int64)
        stp[:stream] = st
        t16 = stp.reshape(nch, 64, 16).transpose(2, 0, 1).reshape(16, nch * 64)
        idxs.append(np.ascontiguousarray(np.tile(t16, (8, 1)).astype(np.int16)))
        dls.append(np.ascontiguousarray(
            dl_stream[c * stream:(c + 1) * stream].reshape(nseg, 128).T))
        ivs.append(np.ascontiguousarray(
            iv_stream[c * stream:(c + 1) * stream].reshape(nseg, 128).T))
    return idxs, dls, ivs


def _fold_weights(p):
    f = lambda k: np.asarray(p[k], np.float32)
    W_ht, b_ht, W_lt, b_lt = f('W_ht'), f('b_ht'), f('W_lt'), f('b_lt')
    Ws_h, Wn_h, b_h = f('Ws_h'), f('Wn_h'), f('b_h')
    Ws_l, Wn_l, Wc_l, b_l = f('Ws_l'), f('Wn_l'), f('Wc_l'), f('b_l')
    Whr, bhr, Wlr, blr = f('Whr'), f('bhr'), f('Wlr'), f('blr')
    Wg, bg = f('Wg'), f('bg')
    Ah = W_ht @ (Ws_h + 0.2 * Whr)
    Al = W_lt @ (Ws_l + 0.2 * Wlr)
    Ag = 2.0 * Wg
    Bh, Bl, Cl = W_ht @ Wn_h, W_lt @ Wn_l, W_lt @ Wc_l
    c_h = b_ht @ (Ws_h + 0.2 * Whr) + b_h + 0.2 * bhr
    c_l = b_lt @ (Ws_l + 0.2 * Wlr) + b_l + 0.2 * blr
    c_r = bg
    u_h, u_l, u_c = b_ht @ Wn_h, b_lt @ Wn_l, b_lt @ Wc_l
    ones = np.ones((OD,), np.float32)

    cw = np.zeros((128, CW_COLS), np.float32)
    rows_b = lambda b: slice(12 * b, 12 * b + 12)
    for b in range(B):
        cw[rows_b(b), CL[f'w1_{b}']] = np.concatenate([Ah, Al], 1)
        cw[96, CL[f'w1_{b}']] = np.concatenate([c_h, c_l])
        cw[rows_b(b), CL[f'w2_{b}']] = np.concatenate([Bh, Bl], 1)
        cw[96, CL[f'w2_{b}']] = np.concatenate([u_h, u_l])
        cw[rows_b(b), CL[f'w3_{b}']] = Cl
        cw[96, CL[f'w3_{b}']] = u_c
        cw[rows_b(b), CL[f'wr_{b}']] = Ag
        cw[96, CL[f'wr_{b}']] = c_r
        cw[rows_b(b), CL[f'wsx_{b}']] = np.stack(
            [Ah @ ones, Al @ ones, Ag @ ones], 1)
        cw[96, CL[f'wsx_{b}']] = [c_h.sum(), c_l.sum(), c_r.sum()]
        cw[rows_b(b), CL[f'wsab_{b}']] = np.stack([Bh @ ones, Bl @ ones], 1)
        cw[96, CL[f'wsab_{b}']] = [u_h.sum(), u_l.sum()]
        cw[rows_b(b), CL[f'wsac_{b}']] = np.stack(
            [np.zeros(T, np.float32), Cl @ ones], 1)
        cw[96, CL[f'wsac_{b}']] = [0.0, u_c.sum()]
    oneshl = np.zeros((128, 2), np.float32)
    oneshl[:64, 0] = 1.0
    oneshl[64:, 1] = 1.0
    cw[:, CL['oneshl']] = oneshl
    cw[0:64, CL['ones64']] = 1.0
    cw[:, CL['gv']] = np.concatenate([f('g_hn'), f('g_ln')])[:, None]
    cw[:, CL['bv']] = np.concatenate([f('b_hn'), f('b_ln')])[:, None]
    cw[0:64, CL['gr']] = (0.1 * f('g_gn'))[:, None]
    cw[0:64, CL['br']] = (0.1 * f('b_gn'))[:, None]
    cw[:, CL['wad']] = (f('Wa')[:, 0] - f('Wa')[:, 1])[:, None]
    cw[0, CL['ehl']] = np.r_[np.ones(64, np.float32), np.zeros(64, np.float32)]
    cw[1, CL['ehl']] = np.r_[np.zeros(64, np.float32), np.ones(64, np.float32)]
    cw[0, CL['ecol0']] = np.r_[np.ones(64, np.float32),
                               np.zeros(64, np.float32)]
    cw[0, CL['ecol1']] = np.r_[np.zeros(64, np.float32),
                               np.ones(64, np.float32)]
    cw[0, CL['onesr']] = 1.0
    cw[:, CL['ident2']] = np.vstack([np.eye(64, dtype=np.float32),
                                     np.eye(64, dtype=np.float32)])
    ba = f('ba')
    cw[0:24, CL['eps']] = EPS
    cw[0:GRP, CL['bad']] = float(ba[0] - ba[1])
    cw[0:GRP, CL['c13']] = 1.3
    return cw, float(ba[0] - ba[1])


# -------------------------------------------------------------- bass program
def _build_program(nseg, nch, ba_diff, repeat=1):
    import concourse.tile as tile
    from concourse import bacc, mybir

    f32 = mybir.dt.float32
    i16 = mybir.dt.int16
    AF = mybir.ActivationFunctionType
    OP = mybir.AluOpType
    SEG_PER_BLK = nseg // NBLK

    nc = bacc.Bacc("TRN2", target_bir_lowering=False, debug=False,
                   enable_asserts=False, num_devices=NCORE)

    xT_d = nc.dram_tensor("xT", [NROWS, 128], f32, kind="ExternalInput")
    rep_d = nc.dram_tensor("rep_tag", [1, 64 * repeat], f32,
                           kind="ExternalInput")
    xsb_d = nc.dram_tensor("xsb", [128, NPCP], f32, kind="ExternalInput")
    cw_d = nc.dram_tensor("cw", [128, CW_COLS], f32, kind="ExternalInput")
    idx_d, dl_d, iv_d = {}, {}, {}
    for g in "bc":
        idx_d[g] = nc.dram_tensor(f"idx_{g}", [128, nch * 64], i16,
                                  kind="ExternalInput")
        dl_d[g] = nc.dram_tensor(f"dl_{g}", [128, nseg], f32,
                                 kind="ExternalInput")
        iv_d[g] = nc.dram_tensor(f"iv_{g}", [128, nseg], f32,
                                 kind="ExternalInput")
    fo_d = nc.dram_tensor("fo", [B, OD, NPCP], f32, kind="ExternalOutput")
    ho_d = nc.dram_tensor("ho", [B, OD, NPCP], f32, kind="ExternalOutput")
    lo_d = nc.dram_tensor("lo", [B, OD, NPCP], f32, kind="ExternalOutput")

    def mmg(mms):
        """Emit matmuls as one PSUM accumulation group.
        mms: list of (out_ap, lhsT_ap, rhs_ap, tile_position)."""
        nmm = len(mms)
        for i, (out, lhsT, rhs, tp) in enumerate(mms):
            nc.tensor.matmul(out, lhsT, rhs, start=(i == 0),
                             stop=(i == nmm - 1), skip_group_check=True,
                             tile_position=tp)

    with tile.TileContext(nc) as tc:
        with (
            tc.tile_pool(name="const", bufs=1) as cpool,
        ):
            cw = cpool.tile([128, CW_COLS], f32, tag="cw")
            nc.sync.dma_start(cw[:], cw_d.ap())

            def Wf(name, rows=128):
                return cw[0:rows, CL[name]]

            rep_t = cpool.tile([1, 64], f32, tag="rep")
            nc.sync.dma_start(rep_t[:], rep_d.ap()[:, 0:64])
            xsb = cpool.tile([128, NPCP], f32, tag="xsb")
            nc.sync.dma_start(xsb[:], xsb_d.ap())
            iota_t = cpool.tile([128, 128], f32, tag="iota")
            nc.gpsimd.iota(iota_t[:], pattern=[[1, 128]], base=0,
                           channel_multiplier=0,
                           allow_small_or_imprecise_dtypes=True)
            idx_t, dl_t, iv_t, aggT = {}, {}, {}, {}
            for g in "bc":
                idx_t[g] = cpool.tile([128, nch * 64], i16, tag=f"idx{g}",
                                      name=f"idx{g}")
                nc.sync.dma_start(idx_t[g][:], idx_d[g].ap())
                dl_t[g] = cpool.tile([128, nseg], f32, tag=f"dl{g}",
                                     name=f"dlt{g}")
                nc.sync.dma_start(dl_t[g][:], dl_d[g].ap())
                iv_t[g] = cpool.tile([128, nseg], f32, tag=f"iv{g}",
                                     name=f"ivt{g}")
                nc.sync.dma_start(iv_t[g][:], iv_d[g].ap())
                aggT[g] = cpool.tile([128, NPCP], f32, tag=f"agg{g}",
                                     name=f"aggT{g}")
                nc.gpsimd.memset(aggT[g][:], 0.0)

            # ---- gather + one-hot segment-sum (per graph) ----
            for _rep in range(repeat):
              with (
                  tc.tile_pool(name="gat", bufs=4) as gpool,
                  tc.tile_pool(name="oh", bufs=6) as ohpool,
                  tc.tile_pool(name="aggps", bufs=2, space="PSUM") as aggps,
              ):
                for g in "bc":
                  ps_blk = None
                  for k in range(nch):
                      gt = gpool.tile([128, 8 * 128], f32, tag="g")
                      gt3 = gt[:].rearrange("p (c e) -> p c e", e=128)
                      nc.gpsimd.dma_gather(
                          gt3, xT_d.ap(),
                          idx_t[g][:, k * 64:(k + 1) * 64],
                          num_idxs=1024, num_idxs_reg=1024, elem_size=128)
                      for c in range(8):
                          s = k * 8 + c
                          if s >= nseg:
                              break
                          r = s % SEG_PER_BLK
                          j = s // SEG_PER_BLK
                          if r == 0:
                              ps_blk = aggps.tile([128, 128], f32, tag="agg")
                          oh = ohpool.tile([128, 128], f32, tag="oh")
                          nc.vector.tensor_scalar(
                              oh[:], iota_t[:],
                              dl_t[g][:, s:s + 1], iv_t[g][:, s:s + 1],
                              OP.is_equal, OP.mult)
                          nc.tensor.matmul(
                              ps_blk[0:97, :], gt3[:, c, 0:97], oh[:],
                              start=(r == 0), stop=(r == SEG_PER_BLK - 1),
                              skip_group_check=True, tile_position=(0, 0))
                          if r == SEG_PER_BLK - 1:
                              nc.scalar.activation(
                                  aggT[g][0:97, j * 128:(j + 1) * 128],
                                  ps_blk[0:97, :], AF.Copy)

              # ---- dense phase in groups of GRP pairs ----
              pairs = [(b, c0, kl) for b in range(B) for (c0, kl) in CHUNKS]
              with (
                  tc.tile_pool(name="mainps", bufs=2, space="PSUM") as mainps,
                  tc.tile_pool(name="statps", bufs=2, space="PSUM") as statps,
                  tc.tile_pool(name="ebc", bufs=2, space="PSUM") as ebcps,
                  tc.tile_pool(name="shl", bufs=GRP + 2) as shlpool,
                  tc.tile_pool(name="sr", bufs=GRP + 2) as srpool,
                  tc.tile_pool(name="hla", bufs=GRP + 2) as hlapool,
                  tc.tile_pool(name="sq", bufs=2) as sqpool,
                  tc.tile_pool(name="ssb", bufs=2) as ssbpool,
                  tc.tile_pool(name="stg", bufs=2) as stgpool,
                  tc.tile_pool(name="tmp", bufs=2) as tmppool,
                  tc.tile_pool(name="stat", bufs=1) as statpool,
                  tc.tile_pool(name="smax", bufs=1) as smaxpool,
              ):
               for grp in range(NGRP):
                  gpairs = list(enumerate(pairs[grp * GRP:(grp + 1) * GRP]))
                  st1 = statpool.tile([3 * GRP, 512], f32, tag="st1")
                  st2 = statpool.tile([3 * GRP, 512], f32, tag="st2")
                  sdt = smaxpool.tile([GRP, 512], f32, tag="sdt")
                  nc.gpsimd.memset(st1[:], 0.0)
                  nc.gpsimd.memset(st2[:], 1.0)
                  nc.gpsimd.memset(sdt[:], 0.0)
                  shl_t, sr_t, hla_t = {}, {}, {}

                  for q, (b, c0, kl) in gpairs:
                      xr = xsb[:, c0:c0 + kl]
                      ab = aggT['b'][:, c0:c0 + kl]
                      ac = aggT['c'][:, c0:c0 + kl]

                      phl = mainps.tile([128, 512], f32, tag="phl")
                      mmg([(phl[:, 0:kl], Wf(f'w1_{b}'), xr, (0, 0)),
                           (phl[:, 0:kl], Wf(f'w2_{b}'), ab, (0, 0)),
                           (phl[64:128, 0:kl], Wf(f'w3_{b}'), ac, (0, 64))])
                      pres = mainps.tile([64, 512], f32, tag="pres")
                      mmg([(pres[:, 0:kl], Wf(f'wr_{b}'), xr, (0, 0))])

                      sh = shlpool.tile([128, 512], f32, tag="shl")
                      shl_t[q] = sh
                      nc.scalar.activation(sh[:, 0:kl], phl[:, 0:kl], AF.Copy)
                      sr = srpool.tile([64, 512], f32, tag="sr",
                                       name=f"sr{q}")
                      sr_t[q] = sr
                      nc.scalar.activation(sr[0:64, 0:kl],
                                           pres[:, 0:kl], AF.Copy)
                      sq = sqpool.tile([128, 512], f32, tag="sq")
                      nc.scalar.activation(sq[:, 0:kl], sh[:, 0:kl], AF.Square)
                      sqr = sqpool.tile([64, 512], f32, tag="sqr")
                      nc.scalar.activation(sqr[:, 0:kl],
                                           sr[0:64, 0:kl], AF.Square)

                      # stats psum: sums@0:3, sumsq_hl@32:34, sumsq_r@64:65
                      S = statps.tile([65, 512], f32, tag="S")
                      mmg([(S[0:3, 0:kl], Wf(f'wsx_{b}'), xr, (0, 0)),
                           (S[0:2, 0:kl], Wf(f'wsab_{b}'), ab, (0, 0)),
                           (S[0:2, 0:kl], Wf(f'wsac_{b}'), ac, (0, 0))])
                      mmg([(S[32:34, 0:kl], Wf('oneshl'), sq[:, 0:kl],
                            (0, 32))])
                      mmg([(S[64:65, 0:kl], Wf('ones64', 64), sqr[:, 0:kl],
                            (0, 64))])
                      ssb = ssbpool.tile([65, 512], f32, tag="ssb")
                      nc.scalar.activation(ssb[:, 0:kl], S[:, 0:kl], AF.Copy)
                      nc.sync.dma_start(st1[3 * q:3 * q + 3, 0:kl],
                                        ssb[0:3, 0:kl])
                      nc.sync.dma_start(st2[3 * q:3 * q + 2, 0:kl],
                                        ssb[32:34, 0:kl])
                      nc.sync.dma_start(st2[3 * q + 2:3 * q + 3, 0:kl],
                                        ssb[64:65, 0:kl])

                  # ---- batched stats math ----
                  m_t = statpool.tile([3 * GRP, 512], f32, tag="m")
                  nc.vector.tensor_scalar_mul(m_t[:], st1[:], 1.0 / OD)
                  q_t = statpool.tile([3 * GRP, 512], f32, tag="q")
                  nc.vector.tensor_scalar_mul(q_t[:], st2[:], 1.0 / OD)
                  msq = statpool.tile([3 * GRP, 512], f32, tag="msq")
                  nc.vector.tensor_mul(msq[:], m_t[:], m_t[:])
                  var = statpool.tile([3 * GRP, 512], f32, tag="var")
                  nc.vector.tensor_sub(var[:], q_t[:], msq[:])
                  std = statpool.tile([3 * GRP, 512], f32, tag="std")
                  nc.scalar.activation(std[:], var[:], AF.Sqrt,
                                       bias=Wf('eps', 3 * GRP))
                  rstd = statpool.tile([3 * GRP, 512], f32, tag="rstd")
                  nc.vector.reciprocal(rstd[:], std[:])
                  mrstd = statpool.tile([3 * GRP, 512], f32, tag="mrstd")
                  nc.vector.tensor_mul(mrstd[:], m_t[:], rstd[:])

                  # ---- per-pair LN apply + activations + logit diff ----
                  for q, (b, c0, kl) in gpairs:
                      sh = shl_t[q]
                      rstg = stgpool.tile([2, 512], f32, tag="rstg")
                      nc.sync.dma_start(rstg[:, 0:kl],
                                        rstd[3 * q:3 * q + 2, 0:kl])
                      mstg = stgpool.tile([2, 512], f32, tag="mstg")
                      nc.sync.dma_start(mstg[:, 0:kl],
                                        mrstd[3 * q:3 * q + 2, 0:kl])
                      rbc = ebcps.tile([128, 512], f32, tag="ebc")
                      nc.tensor.matmul(rbc[:, 0:kl], Wf('ehl', 2),
                                       rstg[:, 0:kl],
                                       start=True, stop=True,
                                       skip_group_check=True,
                                       tile_position=(0, 0))
                      mbc = ebcps.tile([128, 512], f32, tag="ebc")
                      nc.tensor.matmul(mbc[:, 0:kl], Wf('ehl', 2),
                                       mstg[:, 0:kl],
                                       start=True, stop=True,
                                       skip_group_check=True,
                                       tile_position=(0, 0))
                      t1 = tmppool.tile([128, 512], f32, tag="t1")
                      nc.vector.tensor_mul(t1[:, 0:kl], sh[:, 0:kl],
                                           rbc[:, 0:kl])
                      t2 = tmppool.tile([128, 512], f32, tag="t2")
                      nc.vector.tensor_sub(t2[:, 0:kl], t1[:, 0:kl],
                                           mbc[:, 0:kl])
                      hla = hlapool.tile([128, 512], f32, tag="hla")
                      hla_t[q] = hla
                      yh = tmppool.tile([64, 512], f32, tag="yh")
                      nc.scalar.activation(yh[:, 0:kl], t2[0:64, 0:kl],
                                           AF.Identity, bias=Wf('bv')[0:64, :],
                                           scale=Wf('gv')[0:64, :])
                      nc.vector.scalar_tensor_tensor(
                          hla[0:64, 0:kl], yh[:, 0:kl], 0.1, yh[:, 0:kl],
                          OP.mult, OP.max)
                      nc.scalar.activation(hla[64:128, 0:kl], t2[64:128, 0:kl],
                                           AF.Gelu, bias=Wf('bv')[64:128, :],
                                           scale=Wf('gv')[64:128, :])
                      nc.sync.dma_start(ho_d.ap()[b, :, c0:c0 + kl],
                                        hla[0:64, 0:kl])
                      nc.sync.dma_start(lo_d.ap()[b, :, c0:c0 + kl],
                                        hla[64:128, 0:kl])
                      sd = statps.tile([1, 512], f32, tag="S")
                      nc.tensor.matmul(sd[:, 0:kl], Wf('wad'), hla[:, 0:kl],
                                       start=True, stop=True,
                                       skip_group_check=True,
                                       tile_position=(0, 0))
                      sdb = ssbpool.tile([1, 512], f32, tag="sdb")
                      nc.scalar.activation(sdb[:, 0:kl], sd[:, 0:kl], AF.Copy)
                      nc.sync.dma_start(sdt[q:q + 1, 0:kl], sdb[:, 0:kl])

                  # ---- batched 2-way softmax ----
                  a0 = smaxpool.tile([GRP, 512], f32, tag="a0")
                  nc.scalar.activation(a0[:], sdt[:], AF.Sigmoid,
                                       bias=Wf('bad', GRP))
                  w0 = smaxpool.tile([GRP, 512], f32, tag="w0")
                  nc.vector.tensor_scalar_add(w0[:], a0[:], 0.3)
                  w1_ = smaxpool.tile([GRP, 512], f32, tag="w1_")
                  nc.scalar.activation(w1_[:], a0[:], AF.Identity,
                                       bias=Wf('c13', GRP), scale=-1.0)

                  # ---- per-pair fusion + residual + output ----
                  for q, (b, c0, kl) in gpairs:
                      hla = hla_t[q]
                      sr = sr_t[q]
                      w0s = stgpool.tile([1, 512], f32, tag="w0s")
                      nc.sync.dma_start(w0s[:, 0:kl], w0[q:q + 1, 0:kl])
                      w1s = stgpool.tile([1, 512], f32, tag="w1s")
                      nc.sync.dma_start(w1s[:, 0:kl], w1_[q:q + 1, 0:kl])
                      wbc = ebcps.tile([128, 512], f32, tag="ebc")
                      nc.tensor.matmul(wbc[:, 0:kl], Wf('ecol0', 1),
                                       w0s[:, 0:kl], start=True,
                                       stop=False, skip_group_check=True,
                                       tile_position=(0, 0))
                      nc.tensor.matmul(wbc[:, 0:kl], Wf('ecol1', 1),
                                       w1s[:, 0:kl], start=False,
                                       stop=True, skip_group_check=True,
                                       tile_position=(0, 0))
                      f1 = tmppool.tile([128, 512], f32, tag="f1")
                      nc.vector.tensor_mul(f1[:, 0:kl], hla[:, 0:kl],
                                           wbc[:, 0:kl])
                      rrs = stgpool.tile([1, 512], f32, tag="rrs")
                      nc.sync.dma_start(rrs[:, 0:kl],
                                        rstd[3 * q + 2:3 * q + 3, 0:kl])
                      rms = stgpool.tile([1, 512], f32, tag="rms")
                      nc.sync.dma_start(rms[:, 0:kl],
                                        mrstd[3 * q + 2:3 * q + 3, 0:kl])
                      rr = ebcps.tile([64, 512], f32, tag="ebc")
                      nc.tensor.matmul(rr[:, 0:kl], Wf('onesr', 1),
                                       rrs[:, 0:kl],
                                       start=True, stop=True,
                                       skip_group_check=True,
                                       tile_position=(0, 0))
                      rm = ebcps.tile([64, 512], f32, tag="ebc")
                      nc.tensor.matmul(rm[:, 0:kl], Wf('onesr', 1),
                                       rms[:, 0:kl],
                                       start=True, stop=True,
                                       skip_group_check=True,
                                       tile_position=(0, 0))
                      u1 = tmppool.tile([64, 512], f32, tag="u1")
                      nc.vector.tensor_mul(u1[:, 0:kl], sr[0:64, 0:kl],
                                           rr[:, 0:kl])
                      u2 = tmppool.tile([64, 512], f32, tag="u2")
                      nc.vector.tensor_sub(u2[:, 0:kl], u1[:, 0:kl],
                                           rm[:, 0:kl])
                      resa = tmppool.tile([64, 512], f32, tag="resa")
                      nc.scalar.activation(resa[:, 0:kl], u2[:, 0:kl],
                                           AF.Identity, bias=Wf('br')[0:64, :],
                                           scale=Wf('gr')[0:64, :])
                      f2 = ebcps.tile([64, 512], f32, tag="ebc")
                      nc.tensor.matmul(f2[:, 0:kl], Wf('ident2'), f1[:, 0:kl],
                                       start=True, stop=True,
                                       skip_group_check=True,
                                       tile_position=(0, 0))
                      f3 = tmppool.tile([64, 512], f32, tag="f3")
                      nc.vector.tensor_add(f3[:, 0:kl], f2[:, 0:kl],
                                           resa[:, 0:kl])
                      nc.sync.dma_start(fo_d.ap()[b, :, c0:c0 + kl],
                                        f3[:, 0:kl])
    nc.finalize()
    return nc


# ------------------------------------------------------------------- runner
class _SpmdRunner:
    def __init__(self, nc, n_cores=NCORE):
        import jax
        from jax.sharding import Mesh, PartitionSpec
        from jax.experimental.shard_map import shard_map
        from concourse import mybir
        from concourse.bass2jax import (_bass_exec_p, install_neuronx_cc_hook,
                                        partition_id_tensor)
        install_neuronx_cc_hook()
        self.jax = jax
        self.n_cores = n_cores
        partition_name = (nc.partition_id_tensor.name
                          if nc.partition_id_tensor else None)
        in_names, out_names, out_avals, zero_outs = [], [], [], []
        for alloc in nc.m.functions[0].allocations:
            if not isinstance(alloc, mybir.MemoryLocationSet):
                continue
            name = alloc.memorylocations[0].name
            if alloc.kind == "ExternalInput":
                if name != partition_name:
                    in_names.append(name)
            elif alloc.kind == "ExternalOutput":
                out_names.append(name)
                shape = tuple(alloc.tensor_shape)
                dtype = mybir.dt.np(alloc.dtype)
                out_avals.append(jax.core.ShapedArray(shape, dtype))
                zero_outs.append(np.zeros(shape, dtype))
        self.in_names, self.out_names = in_names, out_names
        self.out_avals = out_avals
        n_params, n_outs = len(in_names), len(out_avals)
        all_in = list(in_names) + list(out_names)
        if partition_name is not None:
            all_in.append(partition_name)

        def _body(*args):
            operands = list(args)
            if partition_name is not None:
                operands.append(partition_id_tensor())
            outs = _bass_exec_p.bind(
                *operands, out_avals=tuple(out_avals),
                in_names=tuple(all_in), out_names=tuple(out_names),
                lowering_input_output_aliases=(),
                sim_require_finite=True, sim_require_nnan=True, nc=nc)
            return tuple(outs)

        devices = jax.devices()[:n_cores]
        mesh = Mesh(np.asarray(devices), ("core",))
        self.mesh = mesh
        in_specs = (PartitionSpec("core"),) * (n_params + n_outs)
        out_specs = (PartitionSpec("core"),) * n_outs
        self.fn = jax.jit(
            shard_map(_body, mesh=mesh, in_specs=in_specs,
                      out_specs=out_specs, check_rep=False),
            keep_unused=True)
        self._concat_zeros = [
            np.zeros((n_cores * z.shape[0], *z.shape[1:]), z.dtype)
            for z in zero_outs]

    def prepare(self, in_maps):
        from jax.sharding import NamedSharding, PartitionSpec
        n = self.n_cores
        per_core = [[np.ascontiguousarray(m[name]) for name in self.in_names]
                    for m in in_maps]
        concat_in = [np.concatenate([per_core[c][i] for c in range(n)], axis=0)
                     for i in range(len(self.in_names))]
        args = concat_in + self._concat_zeros
        # Pre-shard along dim 0 so each run() call dispatches the kernel
        # directly instead of inserting per-call resharding copies.
        sh = NamedSharding(self.mesh, PartitionSpec("core"))
        return [self.jax.device_put(a, sh) for a in args]

    def run(self, args):
        outs = self.fn(*args)
        self.jax.block_until_ready(outs)
        return outs

    def split_outs(self, outs):
        res = []
        for c in range(self.n_cores):
            d = {}
            for i, name in enumerate(self.out_names):
                d[name] = np.asarray(outs[i]).reshape(
                    self.n_cores, *self.out_avals[i].shape)[c]
            res.append(d)
        return res


# -------------------------------------------------------------------- entry
def _get(inputs):
    gb = _prep_graph(inputs['edge_index'])
    gc = _prep_graph(inputs['causal_edge_index'])
    b_pad = max(128, -(-int(max(gb['counts'].max(), gc['counts'].max()))
                     // 128) * 128)
    stream = NBLK * b_pad
    nseg = stream // 128
    nch = -(-stream // 1024)
    cw, ba_diff = _fold_weights(inputs)
    key = (b_pad, nseg, nch, round(ba_diff, 9), REPEAT)
    if key not in _cache:
        nc = _build_program(nseg, nch, ba_diff, REPEAT)
        _cache[key] = _SpmdRunner(nc)
    return _cache[key], gb, gc, b_pad, nseg, nch, cw


def make_in_maps(inputs):
    runner, gb, gc, b_pad, nseg, nch, cw = _get(inputs)
    x = np.asarray(inputs['x'], np.float32)
    xflat = np.zeros((128, N), np.float32)
    xflat[0:96] = x.reshape(96, N)
    xflat[96] = 1.0
    xT = np.zeros((NROWS, 128), np.float32)
    xT[:N, 0:97] = xflat[0:97].T
    idx_b, dl_b, iv_b = _build_streams(gb, b_pad, nseg, nch)
    idx_c, dl_c, iv_c = _build_streams(gc, b_pad, nseg, nch)
    in_maps = []
    for c in range(NCORE):
        xs = np.zeros((128, NPCP), np.float32)
        xs[:, 0:NPC] = xflat[:, c * NPC:(c + 1) * NPC]
        in_maps.append({
            'xT': xT, 'xsb': xs, 'cw': cw,
            'rep_tag': np.zeros((1, 64 * REPEAT), np.float32),
            'idx_b': idx_b[c], 'dl_b': dl_b[c], 'iv_b': iv_b[c],
            'idx_c': idx_c[c], 'dl_c': dl_c[c], 'iv_c': iv_c[c],
        })
    return runner, in_maps


def kernel(**inputs):
    runner, in_maps = make_in_maps(inputs)
    args = runner.prepare(in_maps)
    outs = runner.run(args)
    res = runner.split_outs(outs)
    fused = np.empty((B, OD, N), np.float32)
    high = np.empty((B, OD, N), np.float32)
    low = np.empty((B, OD, N), np.float32)
    for c in range(NCORE):
        sl = slice(c * NPC, (c + 1) * NPC)
        fused[:, :, sl] = res[c]['fo'][:, :, 0:NPC]
        high[:, :, sl] = res[c]['ho'][:, :, 0:NPC]
        low[:, :, sl] = res[c]['lo'][:, :, 0:NPC]
    return fused, high, low



# revision 6
# speedup vs baseline: 165.4671x; 1.3130x over previous
"""Trainium2 Bass kernel for nn_DWTEnhancedSTGCN (B=8, T=12, N=10000, E=160000).

Strategy (N-sharded over 8 NeuronCores):
  - Each core owns 1250 dst-nodes for ALL 8 batch elements; edges are shared
    across the batch, so each edge's 96 batch-features (8b x 12t) are gathered
    ONCE per core via dma_gather (256B bf16 rows from an xT scratch in HBM).
  - Aggregation = mean over in-edges. Linearity lets us aggregate the 12-dim
    x features instead of 64-dim hidden features; mean normalization (invdeg)
    is folded into host-precomputed one-hot segment matrices
       onehot[e, j] = (j == dstloc[e]) * invdeg[dst[e]]   (bf16, uploaded),
    then aggT_block += G_chunk.T @ onehot on the PE.
  - Dense phase in [feature, node] layout with K=128 zero-padded weight
    blocks per batch (biases ride on the all-ones row 96). All matmuls and
    elementwise work run in bf16 (f32 PSUM accumulation); LayerNorm stats
    math stays f32. Activations on ACT; fusion on DVE.
Host does only: sharding/reshapes, integer index-stream building, and
parameter-only weight folding (incl. the graph-structure one-hot streams).
All FP math on x runs on device.
"""
import sys
import numpy as np
from ml_dtypes import bfloat16

sys.path.insert(0, '/opt/trn_rl_repo')

B, T, N = 8, 12, 10000
OD = 64
NCORE = 8
NPC = N // NCORE          # 1250 nodes per core
NPCP = 1280               # padded local node count (10 blocks of 128)
NBLK = NPCP // 128
EPS = 1e-5
PADROW = N                # all-zero row in xT used by padding gather idxs
NROWS = N + 16
CHUNKS = [(0, 512), (512, 512), (1024, 256)]
NPAIR = B * len(CHUNKS)   # 24
GRP = 8                   # pairs per stats/softmax group
NGRP = NPAIR // GRP
REPEAT = 1                # in-kernel repetition (timing mode)

# column layout of the packed bf16 constant tile cwb [128, CWB_COLS]
CLB = {}
_cb = 0
def _clb(name, cols):
    global _cb
    CLB[name] = slice(_cb, _cb + cols)
    _cb += cols
for _b in range(B):
    _clb(f'w1_{_b}', 128)   # [A_h|A_l] at rows 12b..12b+11, biases at row 96
    _clb(f'w2_{_b}', 128)   # [B_h|B_l] + u at row 96
    _clb(f'w3_{_b}', 64)    # C_l + u_c at row 96
    _clb(f'wr_{_b}', 64)    # Ag + c_r at row 96
    _clb(f'wsx_{_b}', 3)    # col sums of (high, low, res) x-parts (+bias sums)
    _clb(f'wsab_{_b}', 2)
    _clb(f'wsac_{_b}', 2)
_clb('oneshl', 2)
_clb('ones64', 1)
_clb('wad', 1)
_clb('ehl', 128)
_clb('ecol0', 128)
_clb('ecol1', 128)
_clb('onesr', 64)
_clb('ident2', 64)
CWB_COLS = -(-_cb // 64) * 64

# f32 constant tile cwf [128, CWF_COLS] (ACT bias columns for f32 math)
CLF = {}
_cf = 0
def _clf(name, cols):
    global _cf
    CLF[name] = slice(_cf, _cf + cols)
    _cf += cols
_clf('eps', 1)
_clf('bad', 1)
_clf('c13', 1)
_clf('gv', 1)
_clf('bv', 1)
_clf('gr', 1)
_clf('br', 1)
CWF_COLS = 64

_cache = {}


# ----------------------------------------------------------------- host prep
def _prep_graph(edge_index):
    src = np.asarray(edge_index[0]).astype(np.int64).ravel()
    dst = np.asarray(edge_index[1]).astype(np.int64).ravel()
    deg = np.bincount(dst, minlength=N)
    invdeg = (1.0 / np.maximum(deg, 1)).astype(np.float32)
    order = np.argsort(dst, kind='stable')
    s_s, d_s = src[order], dst[order]
    core = d_s // NPC
    local = d_s - core * NPC
    blk = local >> 7
    dstloc = local & 127
    binid = core * NBLK + blk
    counts = np.bincount(binid, minlength=NCORE * NBLK)
    return dict(s=s_s, d=d_s, core=core, binid=binid, dstloc=dstloc,
                blk=blk, counts=counts, invdeg=invdeg)


def _build_streams(g, b_pad, nseg, nch):
    stream = NBLK * b_pad
    starts = np.zeros(NCORE * NBLK, np.int64)
    np.cumsum(g['counts'][:-1], out=starts[1:])
    rank = np.arange(len(g['s'])) - starts[g['binid']]
    pos = g['core'] * stream + g['blk'] * b_pad + rank
    src_stream = np.full(NCORE * stream, PADROW, np.int64)
    dl_stream = np.full(NCORE * stream, -1, np.int64)
    iv_stream = np.zeros(NCORE * stream, np.float32)
    src_stream[pos] = g['s']
    dl_stream[pos] = g['dstloc']
    iv_stream[pos] = g['invdeg'][g['d']]
    seg_i = (np.arange(stream) // 128).astype(np.int64)
    row_i = (np.arange(stream) % 128).astype(np.int64)
    idxs, ohs = [], []
    for c in range(NCORE):
        st = src_stream[c * stream:(c + 1) * stream]
        stp = np.full(nch * 1024, PADROW, np.int64)
        stp[:stream] = st
        t16 = stp.reshape(nch, 64, 16).transpose(2, 0, 1).reshape(16, nch * 64)
        idxs.append(np.ascontiguousarray(np.tile(t16, (8, 1)).astype(np.int16)))
        dl = dl_stream[c * stream:(c + 1) * stream]
        iv = iv_stream[c * stream:(c + 1) * stream]
        valid = dl >= 0
        oh = np.zeros((128, nseg * 128), np.float32)
        oh[row_i[valid], seg_i[valid] * 128 + dl[valid]] = iv[valid]
        ohs.append(np.ascontiguousarray(oh.astype(bfloat16)))
    return idxs, ohs


def _fold_weights(p):
    f = lambda k: np.asarray(p[k], np.float32)
    W_ht, b_ht, W_lt, b_lt = f('W_ht'), f('b_ht'), f('W_lt'), f('b_lt')
    Ws_h, Wn_h, b_h = f('Ws_h'), f('Wn_h'), f('b_h')
    Ws_l, Wn_l, Wc_l, b_l = f('Ws_l'), f('Wn_l'), f('Wc_l'), f('b_l')
    Whr, bhr, Wlr, blr = f('Whr'), f('bhr'), f('Wlr'), f('blr')
    Wg, bg = f('Wg'), f('bg')
    Ah = W_ht @ (Ws_h + 0.2 * Whr)
    Al = W_lt @ (Ws_l + 0.2 * Wlr)
    Ag = 2.0 * Wg
    Bh, Bl, Cl = W_ht @ Wn_h, W_lt @ Wn_l, W_lt @ Wc_l
    c_h = b_ht @ (Ws_h + 0.2 * Whr) + b_h + 0.2 * bhr
    c_l = b_lt @ (Ws_l + 0.2 * Wlr) + b_l + 0.2 * blr
    c_r = bg
    u_h, u_l, u_c = b_ht @ Wn_h, b_lt @ Wn_l, b_lt @ Wc_l
    ones = np.ones((OD,), np.float32)

    cw = np.zeros((128, CWB_COLS), np.float32)
    rows_b = lambda b: slice(12 * b, 12 * b + 12)
    for b in range(B):
        cw[rows_b(b), CLB[f'w1_{b}']] = np.concatenate([Ah, Al], 1)
        cw[96, CLB[f'w1_{b}']] = np.concatenate([c_h, c_l])
        cw[rows_b(b), CLB[f'w2_{b}']] = np.concatenate([Bh, Bl], 1)
        cw[96, CLB[f'w2_{b}']] = np.concatenate([u_h, u_l])
        cw[rows_b(b), CLB[f'w3_{b}']] = Cl
        cw[96, CLB[f'w3_{b}']] = u_c
        cw[rows_b(b), CLB[f'wr_{b}']] = Ag
        cw[96, CLB[f'wr_{b}']] = c_r
        cw[rows_b(b), CLB[f'wsx_{b}']] = np.stack(
            [Ah @ ones, Al @ ones, Ag @ ones], 1)
        cw[96, CLB[f'wsx_{b}']] = [c_h.sum(), c_l.sum(), c_r.sum()]
        cw[rows_b(b), CLB[f'wsab_{b}']] = np.stack([Bh @ ones, Bl @ ones], 1)
        cw[96, CLB[f'wsab_{b}']] = [u_h.sum(), u_l.sum()]
        cw[rows_b(b), CLB[f'wsac_{b}']] = np.stack(
            [np.zeros(T, np.float32), Cl @ ones], 1)
        cw[96, CLB[f'wsac_{b}']] = [0.0, u_c.sum()]
    oneshl = np.zeros((128, 2), np.float32)
    oneshl[:64, 0] = 1.0
    oneshl[64:, 1] = 1.0
    cw[:, CLB['oneshl']] = oneshl
    cw[0:64, CLB['ones64']] = 1.0
    cw[:, CLB['wad']] = (f('Wa')[:, 0] - f('Wa')[:, 1])[:, None]
    cw[0, CLB['ehl']] = np.r_[np.ones(64, np.float32), np.zeros(64, np.float32)]
    cw[1, CLB['ehl']] = np.r_[np.zeros(64, np.float32), np.ones(64, np.float32)]
    cw[0, CLB['ecol0']] = np.r_[np.ones(64, np.float32),
                                np.zeros(64, np.float32)]
    cw[0, CLB['ecol1']] = np.r_[np.zeros(64, np.float32),
                                np.ones(64, np.float32)]
    cw[0, CLB['onesr']] = 1.0
    cw[:, CLB['ident2']] = np.vstack([np.eye(64, dtype=np.float32),
                                      np.eye(64, dtype=np.float32)])
    cwb = cw.astype(bfloat16)

    cwf = np.zeros((128, CWF_COLS), np.float32)
    ba = f('ba')
    cwf[0:3 * GRP, CLF['eps']] = EPS
    cwf[0:GRP, CLF['bad']] = float(ba[0] - ba[1])
    cwf[0:GRP, CLF['c13']] = 1.3
    cwf[:, CLF['gv']] = np.concatenate([f('g_hn'), f('g_ln')])[:, None]
    cwf[:, CLF['bv']] = np.concatenate([f('b_hn'), f('b_ln')])[:, None]
    cwf[0:64, CLF['gr']] = (0.1 * f('g_gn'))[:, None]
    cwf[0:64, CLF['br']] = (0.1 * f('b_gn'))[:, None]
    return cwb, cwf, float(ba[0] - ba[1])


# -------------------------------------------------------------- bass program
def _build_program(nseg, nch, ba_diff, repeat=1):
    import concourse.tile as tile
    from concourse import bacc, mybir

    f32 = mybir.dt.float32
    bf16 = mybir.dt.bfloat16
    i16 = mybir.dt.int16
    AF = mybir.ActivationFunctionType
    OP = mybir.AluOpType
    SEG_PER_BLK = nseg // NBLK

    nc = bacc.Bacc("TRN2", target_bir_lowering=False, debug=False,
                   enable_asserts=False, num_devices=NCORE)

    xT_d = nc.dram_tensor("xT", [NROWS, 128], bf16, kind="ExternalInput")
    rep_d = nc.dram_tensor("rep_tag", [1, 64 * repeat], f32,
                           kind="ExternalInput")
    xsb_d = nc.dram_tensor("xsb", [128, NPCP], bf16, kind="ExternalInput")
    cwb_d = nc.dram_tensor("cwb", [128, CWB_COLS], bf16, kind="ExternalInput")
    cwf_d = nc.dram_tensor("cwf", [128, CWF_COLS], f32, kind="ExternalInput")
    idx_d, oh_d = {}, {}
    for g in "bc":
        idx_d[g] = nc.dram_tensor(f"idx_{g}", [128, nch * 64], i16,
                                  kind="ExternalInput")
        oh_d[g] = nc.dram_tensor(f"oh_{g}", [128, nseg * 128], bf16,
                                 kind="ExternalInput")
    fo_d = nc.dram_tensor("fo", [B, OD, NPCP], f32, kind="ExternalOutput")
    ho_d = nc.dram_tensor("ho", [B, OD, NPCP], f32, kind="ExternalOutput")
    lo_d = nc.dram_tensor("lo", [B, OD, NPCP], f32, kind="ExternalOutput")

    def mmg(mms):
        """Emit matmuls as one PSUM accumulation group.
        mms: list of (out_ap, lhsT_ap, rhs_ap, tile_position)."""
        nmm = len(mms)
        for i, (out, lhsT, rhs, tp) in enumerate(mms):
            nc.tensor.matmul(out, lhsT, rhs, start=(i == 0),
                             stop=(i == nmm - 1), skip_group_check=True,
                             tile_position=tp)

    with tile.TileContext(nc) as tc:
        with (
            tc.tile_pool(name="const", bufs=1) as cpool,
        ):
            cwb = cpool.tile([128, CWB_COLS], bf16, tag="cwb")
            nc.sync.dma_start(cwb[:], cwb_d.ap())
            cwf = cpool.tile([128, CWF_COLS], f32, tag="cwf")
            nc.sync.dma_start(cwf[:], cwf_d.ap())

            def Wf(name, rows=128):
                return cwb[0:rows, CLB[name]]

            def Ff(name, rows=128):
                return cwf[0:rows, CLF[name]]

            rep_t = cpool.tile([1, 64], f32, tag="rep")
            nc.sync.dma_start(rep_t[:], rep_d.ap()[:, 0:64])
            xsb = cpool.tile([128, NPCP], bf16, tag="xsb")
            nc.sync.dma_start(xsb[:], xsb_d.ap())
            idx_t, oh_t, aggT = {}, {}, {}
            for g in "bc":
                idx_t[g] = cpool.tile([128, nch * 64], i16, tag=f"idx{g}",
                                      name=f"idx{g}")
                nc.sync.dma_start(idx_t[g][:], idx_d[g].ap())
                oh_t[g] = cpool.tile([128, nseg * 128], bf16, tag=f"oh{g}",
                                     name=f"oht{g}")
                # split the one-hot load per dst-block so early segment
                # matmuls don't wait for the whole tile
                for j in range(NBLK):
                    cs = slice(j * SEG_PER_BLK * 128,
                               (j + 1) * SEG_PER_BLK * 128)
                    nc.sync.dma_start(oh_t[g][:, cs], oh_d[g].ap()[:, cs])
                aggT[g] = cpool.tile([128, NPCP], bf16, tag=f"agg{g}",
                                     name=f"aggT{g}")
                nc.gpsimd.memset(aggT[g][:], 0.0)

            # ---- gather + one-hot segment-sum (per graph) ----
            for _rep in range(repeat):
              with (
                  tc.tile_pool(name="gat", bufs=4) as gpool,
                  tc.tile_pool(name="aggps", bufs=2, space="PSUM") as aggps,
              ):
                for g in "bc":
                  ps_blk = None
                  for k in range(nch):
                      gt = gpool.tile([128, 8 * 128], bf16, tag="g")
                      gt3 = gt[:].rearrange("p (c e) -> p c e", e=128)
                      nc.gpsimd.dma_gather(
                          gt3, xT_d.ap(),
                          idx_t[g][:, k * 64:(k + 1) * 64],
                          num_idxs=1024, num_idxs_reg=1024, elem_size=128)
                      for c in range(8):
                          s = k * 8 + c
                          if s >= nseg:
                              break
                          r = s % SEG_PER_BLK
                          j = s // SEG_PER_BLK
                          if r == 0:
                              ps_blk = aggps.tile([128, 128], f32, tag="agg")
                          nc.tensor.matmul(
                              ps_blk[0:97, :], gt3[:, c, 0:97],
                              oh_t[g][:, s * 128:(s + 1) * 128],
                              start=(r == 0), stop=(r == SEG_PER_BLK - 1),
                              skip_group_check=True, tile_position=(0, 0))
                          if r == SEG_PER_BLK - 1:
                              nc.scalar.activation(
                                  aggT[g][0:97, j * 128:(j + 1) * 128],
                                  ps_blk[0:97, :], AF.Copy)

              # ---- dense phase in groups of GRP pairs ----
              pairs = [(b, c0, kl) for b in range(B) for (c0, kl) in CHUNKS]
              with (
                  tc.tile_pool(name="mainps", bufs=2, space="PSUM") as mainps,
                  tc.tile_pool(name="statps", bufs=2, space="PSUM") as statps,
                  tc.tile_pool(name="ebc", bufs=2, space="PSUM") as ebcps,
                  tc.tile_pool(name="shl", bufs=GRP + 2) as shlpool,
                  tc.tile_pool(name="sr", bufs=GRP + 2) as srpool,
                  tc.tile_pool(name="hla", bufs=GRP + 2) as hlapool,
                  tc.tile_pool(name="sq", bufs=2) as sqpool,
                  tc.tile_pool(name="ssb", bufs=2) as ssbpool,
                  tc.tile_pool(name="stg", bufs=2) as stgpool,
                  tc.tile_pool(name="tmp", bufs=2) as tmppool,
                  tc.tile_pool(name="stat", bufs=1) as statpool,
                  tc.tile_pool(name="smax", bufs=1) as smaxpool,
              ):
               for grp in range(NGRP):
                  gpairs = list(enumerate(pairs[grp * GRP:(grp + 1) * GRP]))
                  st1 = statpool.tile([3 * GRP, 512], f32, tag="st1")
                  st2 = statpool.tile([3 * GRP, 512], f32, tag="st2")
                  sdt = smaxpool.tile([GRP, 512], f32, tag="sdt")
                  nc.gpsimd.memset(st1[:], 0.0)
                  nc.gpsimd.memset(st2[:], 1.0)
                  nc.gpsimd.memset(sdt[:], 0.0)
                  shl_t, sr_t, hla_t = {}, {}, {}

                  for q, (b, c0, kl) in gpairs:
                      xr = xsb[:, c0:c0 + kl]
                      ab = aggT['b'][:, c0:c0 + kl]
                      ac = aggT['c'][:, c0:c0 + kl]

                      phl = mainps.tile([128, 512], f32, tag="phl")
                      mmg([(phl[:, 0:kl], Wf(f'w1_{b}'), xr, (0, 0)),
                           (phl[:, 0:kl], Wf(f'w2_{b}'), ab, (0, 0)),
                           (phl[64:128, 0:kl], Wf(f'w3_{b}'), ac, (0, 64))])
                      pres = mainps.tile([64, 512], f32, tag="pres")
                      mmg([(pres[:, 0:kl], Wf(f'wr_{b}'), xr, (0, 0))])

                      sh = shlpool.tile([128, 512], bf16, tag="shl")
                      shl_t[q] = sh
                      nc.scalar.activation(sh[:, 0:kl], phl[:, 0:kl], AF.Copy)
                      sr = srpool.tile([64, 512], bf16, tag="sr",
                                       name=f"sr{q}")
                      sr_t[q] = sr
                      nc.scalar.activation(sr[0:64, 0:kl],
                                           pres[:, 0:kl], AF.Copy)
                      sq = sqpool.tile([128, 512], bf16, tag="sq")
                      nc.scalar.activation(sq[:, 0:kl], sh[:, 0:kl], AF.Square)
                      sqr = sqpool.tile([64, 512], bf16, tag="sqr")
                      nc.scalar.activation(sqr[:, 0:kl],
                                           sr[0:64, 0:kl], AF.Square)

                      # stats psum: sums@0:3, sumsq_hl@32:34, sumsq_r@64:65
                      S = statps.tile([65, 512], f32, tag="S")
                      mmg([(S[0:3, 0:kl], Wf(f'wsx_{b}'), xr, (0, 0)),
                           (S[0:2, 0:kl], Wf(f'wsab_{b}'), ab, (0, 0)),
                           (S[0:2, 0:kl], Wf(f'wsac_{b}'), ac, (0, 0))])
                      mmg([(S[32:34, 0:kl], Wf('oneshl'), sq[:, 0:kl],
                            (0, 32))])
                      mmg([(S[64:65, 0:kl], Wf('ones64', 64), sqr[:, 0:kl],
                            (0, 64))])
                      ssb = ssbpool.tile([65, 512], f32, tag="ssb")
                      nc.scalar.activation(ssb[0:3, 0:kl], S[0:3, 0:kl],
                                           AF.Copy)
                      nc.scalar.activation(ssb[32:34, 0:kl], S[32:34, 0:kl],
                                           AF.Copy)
                      nc.scalar.activation(ssb[64:65, 0:kl], S[64:65, 0:kl],
                                           AF.Copy)
                      nc.sync.dma_start(st1[3 * q:3 * q + 3, 0:kl],
                                        ssb[0:3, 0:kl])
                      nc.sync.dma_start(st2[3 * q:3 * q + 2, 0:kl],
                                        ssb[32:34, 0:kl])
                      nc.sync.dma_start(st2[3 * q + 2:3 * q + 3, 0:kl],
                                        ssb[64:65, 0:kl])

                  # ---- batched stats math (f32) ----
                  m_t = statpool.tile([3 * GRP, 512], f32, tag="m")
                  nc.vector.tensor_scalar_mul(m_t[:], st1[:], 1.0 / OD)
                  q_t = statpool.tile([3 * GRP, 512], f32, tag="q")
                  nc.vector.tensor_scalar_mul(q_t[:], st2[:], 1.0 / OD)
                  msq = statpool.tile([3 * GRP, 512], f32, tag="msq")
                  nc.vector.tensor_mul(msq[:], m_t[:], m_t[:])
                  var = statpool.tile([3 * GRP, 512], f32, tag="var")
                  nc.vector.tensor_sub(var[:], q_t[:], msq[:])
                  std = statpool.tile([3 * GRP, 512], f32, tag="std")
                  nc.scalar.activation(std[:], var[:], AF.Sqrt,
                                       bias=Ff('eps', 3 * GRP))
                  rstd = statpool.tile([3 * GRP, 512], f32, tag="rstd")
                  nc.vector.reciprocal(rstd[:], std[:])
                  mrstd = statpool.tile([3 * GRP, 512], f32, tag="mrstd")
                  nc.vector.tensor_mul(mrstd[:], m_t[:], rstd[:])
                  rstdb = statpool.tile([3 * GRP, 512], bf16, tag="rstdb")
                  nc.scalar.activation(rstdb[:], rstd[:], AF.Copy)
                  mrstdb = statpool.tile([3 * GRP, 512], bf16, tag="mrstdb")
                  nc.scalar.activation(mrstdb[:], mrstd[:], AF.Copy)

                  # ---- per-pair LN apply + activations + logit diff ----
                  for q, (b, c0, kl) in gpairs:
                      sh = shl_t[q]
                      rstg = stgpool.tile([2, 512], bf16, tag="rstg")
                      nc.sync.dma_start(rstg[:, 0:kl],
                                        rstdb[3 * q:3 * q + 2, 0:kl])
                      mstg = stgpool.tile([2, 512], bf16, tag="mstg")
                      nc.sync.dma_start(mstg[:, 0:kl],
                                        mrstdb[3 * q:3 * q + 2, 0:kl])
                      rbc = ebcps.tile([128, 512], f32, tag="ebc")
                      nc.tensor.matmul(rbc[:, 0:kl], Wf('ehl', 2),
                                       rstg[:, 0:kl],
                                       start=True, stop=True,
                                       skip_group_check=True,
                                       tile_position=(0, 0))
                      mbc = ebcps.tile([128, 512], f32, tag="ebc")
                      nc.tensor.matmul(mbc[:, 0:kl], Wf('ehl', 2),
                                       mstg[:, 0:kl],
                                       start=True, stop=True,
                                       skip_group_check=True,
                                       tile_position=(0, 0))
                      t1 = tmppool.tile([128, 512], bf16, tag="t1")
                      nc.vector.tensor_mul(t1[:, 0:kl], sh[:, 0:kl],
                                           rbc[:, 0:kl])
                      t2 = tmppool.tile([128, 512], bf16, tag="t2")
                      nc.vector.tensor_sub(t2[:, 0:kl], t1[:, 0:kl],
                                           mbc[:, 0:kl])
                      hla = hlapool.tile([128, 512], bf16, tag="hla")
                      hla_t[q] = hla
                      yh = tmppool.tile([64, 512], bf16, tag="yh")
                      nc.scalar.activation(yh[:, 0:kl], t2[0:64, 0:kl],
                                           AF.Identity, bias=Ff('bv')[0:64, :],
                                           scale=Ff('gv')[0:64, :])
                      nc.vector.scalar_tensor_tensor(
                          hla[0:64, 0:kl], yh[:, 0:kl], 0.1, yh[:, 0:kl],
                          OP.mult, OP.max)
                      nc.scalar.activation(hla[64:128, 0:kl], t2[64:128, 0:kl],
                                           AF.Gelu, bias=Ff('bv')[64:128, :],
                                           scale=Ff('gv')[64:128, :])
                      hout = tmppool.tile([128, 512], f32, tag="hout")
                      nc.scalar.activation(hout[:, 0:kl], hla[:, 0:kl],
                                           AF.Copy)
                      nc.sync.dma_start(ho_d.ap()[b, :, c0:c0 + kl],
                                        hout[0:64, 0:kl])
                      nc.sync.dma_start(lo_d.ap()[b, :, c0:c0 + kl],
                                        hout[64:128, 0:kl])
                      sd = statps.tile([1, 512], f32, tag="S")
                      nc.tensor.matmul(sd[:, 0:kl], Wf('wad'), hla[:, 0:kl],
                                       start=True, stop=True,
                                       skip_group_check=True,
                                       tile_position=(0, 0))
                      sdb = ssbpool.tile([1, 512], f32, tag="sdb")
                      nc.scalar.activation(sdb[:, 0:kl], sd[:, 0:kl], AF.Copy)
                      nc.sync.dma_start(sdt[q:q + 1, 0:kl], sdb[:, 0:kl])

                  # ---- batched 2-way softmax (f32 in, bf16 staged) ----
                  a0 = smaxpool.tile([GRP, 512], f32, tag="a0")
                  nc.scalar.activation(a0[:], sdt[:], AF.Sigmoid,
                                       bias=Ff('bad', GRP))
                  w0 = smaxpool.tile([GRP, 512], bf16, tag="w0")
                  nc.vector.tensor_scalar_add(w0[:], a0[:], 0.3)
                  w1_ = smaxpool.tile([GRP, 512], bf16, tag="w1_")
                  nc.scalar.activation(w1_[:], a0[:], AF.Identity,
                                       bias=Ff('c13', GRP), scale=-1.0)

                  # ---- per-pair fusion + residual + output ----
                  for q, (b, c0, kl) in gpairs:
                      hla = hla_t[q]
                      sr = sr_t[q]
                      w0s = stgpool.tile([1, 512], bf16, tag="w0s")
                      nc.sync.dma_start(w0s[:, 0:kl], w0[q:q + 1, 0:kl])
                      w1s = stgpool.tile([1, 512], bf16, tag="w1s")
                      nc.sync.dma_start(w1s[:, 0:kl], w1_[q:q + 1, 0:kl])
                      wbc = ebcps.tile([128, 512], f32, tag="ebc")
                      nc.tensor.matmul(wbc[:, 0:kl], Wf('ecol0', 1),
                                       w0s[:, 0:kl], start=True,
                                       stop=False, skip_group_check=True,
                                       tile_position=(0, 0))
                      nc.tensor.matmul(wbc[:, 0:kl], Wf('ecol1', 1),
                                       w1s[:, 0:kl], start=False,
                                       stop=True, skip_group_check=True,
                                       tile_position=(0, 0))
                      f1 = tmppool.tile([128, 512], bf16, tag="f1")
                      nc.vector.tensor_mul(f1[:, 0:kl], hla[:, 0:kl],
                                           wbc[:, 0:kl])
                      rrs = stgpool.tile([1, 512], bf16, tag="rrs")
                      nc.sync.dma_start(rrs[:, 0:kl],
                                        rstdb[3 * q + 2:3 * q + 3, 0:kl])
                      rms = stgpool.tile([1, 512], bf16, tag="rms")
                      nc.sync.dma_start(rms[:, 0:kl],
                                        mrstdb[3 * q + 2:3 * q + 3, 0:kl])
                      rr = ebcps.tile([64, 512], f32, tag="ebc")
                      nc.tensor.matmul(rr[:, 0:kl], Wf('onesr', 1),
                                       rrs[:, 0:kl],
                                       start=True, stop=True,
                                       skip_group_check=True,
                                       tile_position=(0, 0))
                      rm = ebcps.tile([64, 512], f32, tag="ebc")
                      nc.tensor.matmul(rm[:, 0:kl], Wf('onesr', 1),
                                       rms[:, 0:kl],
                                       start=True, stop=True,
                                       skip_group_check=True,
                                       tile_position=(0, 0))
                      u1 = tmppool.tile([64, 512], bf16, tag="u1")
                      nc.vector.tensor_mul(u1[:, 0:kl], sr[0:64, 0:kl],
                                           rr[:, 0:kl])
                      u2 = tmppool.tile([64, 512], bf16, tag="u2")
                      nc.vector.tensor_sub(u2[:, 0:kl], u1[:, 0:kl],
                                           rm[:, 0:kl])
                      resa = tmppool.tile([64, 512], bf16, tag="resa")
                      nc.scalar.activation(resa[:, 0:kl], u2[:, 0:kl],
                                           AF.Identity, bias=Ff('br')[0:64, :],
                                           scale=Ff('gr')[0:64, :])
                      f2 = ebcps.tile([64, 512], f32, tag="ebc")
                      nc.tensor.matmul(f2[:, 0:kl], Wf('ident2'), f1[:, 0:kl],
                                       start=True, stop=True,
                                       skip_group_check=True,
                                       tile_position=(0, 0))
                      f3 = tmppool.tile([64, 512], f32, tag="f3")
                      nc.vector.tensor_add(f3[:, 0:kl], f2[:, 0:kl],
                                           resa[:, 0:kl])
                      nc.sync.dma_start(fo_d.ap()[b, :, c0:c0 + kl],
                                        f3[:, 0:kl])
    nc.finalize()
    return nc


# ------------------------------------------------------------------- runner
class _SpmdRunner:
    def __init__(self, nc, n_cores=NCORE):
        import jax
        from jax.sharding import Mesh, PartitionSpec
        from jax.experimental.shard_map import shard_map
        from concourse import mybir
        from concourse.bass2jax import (_bass_exec_p, install_neuronx_cc_hook,
                                        partition_id_tensor)
        install_neuronx_cc_hook()
        self.jax = jax
        self.n_cores = n_cores
        partition_name = (nc.partition_id_tensor.name
                          if nc.partition_id_tensor else None)
        in_names, out_names, out_avals, zero_outs = [], [], [], []
        for alloc in nc.m.functions[0].allocations:
            if not isinstance(alloc, mybir.MemoryLocationSet):
                continue
            name = alloc.memorylocations[0].name
            if alloc.kind == "ExternalInput":
                if name != partition_name:
                    in_names.append(name)
            elif alloc.kind == "ExternalOutput":
                out_names.append(name)
                shape = tuple(alloc.tensor_shape)
                dtype = mybir.dt.np(alloc.dtype)
                out_avals.append(jax.core.ShapedArray(shape, dtype))
                zero_outs.append(np.zeros(shape, dtype))
        self.in_names, self.out_names = in_names, out_names
        self.out_avals = out_avals
        n_params, n_outs = len(in_names), len(out_avals)
        all_in = list(in_names) + list(out_names)
        if partition_name is not None:
            all_in.append(partition_name)

        def _body(*args):
            operands = list(args)
            if partition_name is not None:
                operands.append(partition_id_tensor())
            outs = _bass_exec_p.bind(
                *operands, out_avals=tuple(out_avals),
                in_names=tuple(all_in), out_names=tuple(out_names),
                lowering_input_output_aliases=(),
                sim_require_finite=True, sim_require_nnan=True, nc=nc)
            return tuple(outs)

        devices = jax.devices()[:n_cores]
        mesh = Mesh(np.asarray(devices), ("core",))
        self.mesh = mesh
        in_specs = (PartitionSpec("core"),) * (n_params + n_outs)
        out_specs = (PartitionSpec("core"),) * n_outs
        self.fn = jax.jit(
            shard_map(_body, mesh=mesh, in_specs=in_specs,
                      out_specs=out_specs, check_rep=False),
            keep_unused=True)
        self._concat_zeros = [
            np.zeros((n_cores * z.shape[0], *z.shape[1:]), z.dtype)
            for z in zero_outs]

    def prepare(self, in_maps):
        from jax.sharding import NamedSharding, PartitionSpec
        n = self.n_cores
        per_core = [[np.ascontiguousarray(m[name]) for name in self.in_names]
                    for m in in_maps]
        concat_in = [np.concatenate([per_core[c][i] for c in range(n)], axis=0)
                     for i in range(len(self.in_names))]
        args = concat_in + self._concat_zeros
        # Pre-shard along dim 0 so each run() call dispatches the kernel
        # directly instead of inserting per-call resharding copies.
        sh = NamedSharding(self.mesh, PartitionSpec("core"))
        return [self.jax.device_put(a, sh) for a in args]

    def run(self, args):
        outs = self.fn(*args)
        self.jax.block_until_ready(outs)
        return outs

    def split_outs(self, outs):
        res = []
        for c in range(self.n_cores):
            d = {}
            for i, name in enumerate(self.out_names):
                d[name] = np.asarray(outs[i]).reshape(
                    self.n_cores, *self.out_avals[i].shape)[c]
            res.append(d)
        return res


# -------------------------------------------------------------------- entry
def _get(inputs):
    gb = _prep_graph(inputs['edge_index'])
    gc = _prep_graph(inputs['causal_edge_index'])
    b_pad = max(128, -(-int(max(gb['counts'].max(), gc['counts'].max()))
                     // 128) * 128)
    stream = NBLK * b_pad
    nseg = stream // 128
    nch = -(-stream // 1024)
    cwb, cwf, ba_diff = _fold_weights(inputs)
    key = (b_pad, nseg, nch, round(ba_diff, 9), REPEAT)
    if key not in _cache:
        nc = _build_program(nseg, nch, ba_diff, REPEAT)
        _cache[key] = _SpmdRunner(nc)
    return _cache[key], gb, gc, b_pad, nseg, nch, cwb, cwf


def make_in_maps(inputs):
    runner, gb, gc, b_pad, nseg, nch, cwb, cwf = _get(inputs)
    x = np.asarray(inputs['x'], np.float32)
    xflat = np.zeros((128, N), np.float32)
    xflat[0:96] = x.reshape(96, N)
    xflat[96] = 1.0
    xT = np.zeros((NROWS, 128), np.float32)
    xT[:N, 0:97] = xflat[0:97].T
    xT16 = xT.astype(bfloat16)
    idx_b, oh_b = _build_streams(gb, b_pad, nseg, nch)
    idx_c, oh_c = _build_streams(gc, b_pad, nseg, nch)
    in_maps = []
    for c in range(NCORE):
        xs = np.zeros((128, NPCP), np.float32)
        xs[:, 0:NPC] = xflat[:, c * NPC:(c + 1) * NPC]
        in_maps.append({
            'xT': xT16, 'xsb': xs.astype(bfloat16), 'cwb': cwb, 'cwf': cwf,
            'rep_tag': np.zeros((1, 64 * REPEAT), np.float32),
            'idx_b': idx_b[c], 'oh_b': oh_b[c],
            'idx_c': idx_c[c], 'oh_c': oh_c[c],
        })
    return runner, in_maps


def kernel(**inputs):
    runner, in_maps = make_in_maps(inputs)
    args = runner.prepare(in_maps)
    outs = runner.run(args)
    res = runner.split_outs(outs)
    fused = np.empty((B, OD, N), np.float32)
    high = np.empty((B, OD, N), np.float32)
    low = np.empty((B, OD, N), np.float32)
    for c in range(NCORE):
        sl = slice(c * NPC, (c + 1) * NPC)
        fused[:, :, sl] = res[c]['fo'][:, :, 0:NPC]
        high[:, :, sl] = res[c]['ho'][:, :, 0:NPC]
        low[:, :, sl] = res[c]['lo'][:, :, 0:NPC]
    return fused, high, low


# revision 10
# speedup vs baseline: 172.6747x; 1.0436x over previous
"""Trainium2 Bass kernel for nn_DWTEnhancedSTGCN (B=8, T=12, N=10000, E=160000).

Strategy (N-sharded over 8 NeuronCores):
  - Each core owns 1250 dst-nodes for ALL 8 batch elements; edges are shared
    across the batch, so each edge's 96 batch-features (8b x 12t) are gathered
    ONCE per core via dma_gather (256B bf16 rows from an xT scratch in HBM).
  - Aggregation = mean over in-edges. Linearity lets us aggregate the 12-dim
    x features instead of 64-dim hidden features; mean normalization (invdeg)
    is folded into host-precomputed one-hot segment matrices
       onehot[e, j] = (j == dstloc[e]) * invdeg[dst[e]]   (bf16, uploaded),
    then aggT_block += G_chunk.T @ onehot on the PE.
  - Dense phase in [feature, node] layout with K=128 zero-padded weight
    blocks per batch (biases ride on the all-ones row 96). All matmuls and
    elementwise work run in bf16 (f32 PSUM accumulation); LayerNorm stats
    math stays f32. Activations on ACT; fusion on DVE.
Host does only: sharding/reshapes, integer index-stream building, and
parameter-only weight folding (incl. the graph-structure one-hot streams).
All FP math on x runs on device.
"""
import sys
import numpy as np
from ml_dtypes import bfloat16

sys.path.insert(0, '/opt/trn_rl_repo')

B, T, N = 8, 12, 10000
OD = 64
NCORE = 8
NPC = N // NCORE          # 1250 nodes per core
NPCP = 1280               # padded local node count (10 blocks of 128)
NBLK = NPCP // 128
EPS = 1e-5
PADROW = N                # all-zero row in xT used by padding gather idxs
NROWS = N + 16
CHUNKS = [(0, 512), (512, 512), (1024, 256)]
NPAIR = B * len(CHUNKS)   # 24
GRP = 8                   # pairs per stats/softmax group
NGRP = NPAIR // GRP
REPEAT = 1                # in-kernel repetition (timing mode)

# column layout of the packed bf16 constant tile cwb [128, CWB_COLS]
CLB = {}
_cb = 0
def _clb(name, cols):
    global _cb
    CLB[name] = slice(_cb, _cb + cols)
    _cb += cols
for _b in range(B):
    _clb(f'w1_{_b}', 128)   # [A_h|A_l] at rows 12b..12b+11, biases at row 96
    _clb(f'w2_{_b}', 128)   # [B_h|B_l] + u at row 96
    _clb(f'w3_{_b}', 64)    # C_l + u_c at row 96
    _clb(f'wr_{_b}', 64)    # Ag + c_r at row 96
    _clb(f'wsx_{_b}', 3)    # col sums of (high, low, res) x-parts (+bias sums)
    _clb(f'wsab_{_b}', 2)
    _clb(f'wsac_{_b}', 2)
_clb('oneshl', 2)
_clb('ones64', 1)
_clb('wad', 1)
_clb('ehl', 128)
_clb('ecol0', 128)
_clb('ecol1', 128)
_clb('onesr', 64)
_clb('ident2', 64)
CWB_COLS = -(-_cb // 64) * 64

# f32 constant tile cwf [128, CWF_COLS] (ACT bias columns for f32 math)
CLF = {}
_cf = 0
def _clf(name, cols):
    global _cf
    CLF[name] = slice(_cf, _cf + cols)
    _cf += cols
_clf('eps', 1)
_clf('bad', 1)
_clf('c13', 1)
_clf('gv', 1)
_clf('bv', 1)
_clf('gr', 1)
_clf('br', 1)
CWF_COLS = 64

_cache = {}


# ----------------------------------------------------------------- host prep
def _prep_graph(edge_index):
    src = np.asarray(edge_index[0]).astype(np.int64).ravel()
    dst = np.asarray(edge_index[1]).astype(np.int64).ravel()
    deg = np.bincount(dst, minlength=N)
    invdeg = (1.0 / np.maximum(deg, 1)).astype(np.float32)
    order = np.argsort(dst, kind='stable')
    s_s, d_s = src[order], dst[order]
    core = d_s // NPC
    local = d_s - core * NPC
    blk = local >> 7
    dstloc = local & 127
    binid = core * NBLK + blk
    counts = np.bincount(binid, minlength=NCORE * NBLK)
    return dict(s=s_s, d=d_s, core=core, binid=binid, dstloc=dstloc,
                blk=blk, counts=counts, invdeg=invdeg)


def _build_streams(g, b_pad, nseg, nch):
    stream = NBLK * b_pad
    starts = np.zeros(NCORE * NBLK, np.int64)
    np.cumsum(g['counts'][:-1], out=starts[1:])
    rank = np.arange(len(g['s'])) - starts[g['binid']]
    pos = g['core'] * stream + g['blk'] * b_pad + rank
    src_stream = np.full(NCORE * stream, PADROW, np.int64)
    dl_stream = np.full(NCORE * stream, -1, np.int64)
    iv_stream = np.zeros(NCORE * stream, np.float32)
    src_stream[pos] = g['s']
    dl_stream[pos] = g['dstloc']
    iv_stream[pos] = g['invdeg'][g['d']]
    seg_i = (np.arange(stream) // 128).astype(np.int64)
    row_i = (np.arange(stream) % 128).astype(np.int64)
    idxs, ohs = [], []
    for c in range(NCORE):
        st = src_stream[c * stream:(c + 1) * stream]
        stp = np.full(nch * 1024, PADROW, np.int64)
        stp[:stream] = st
        t16 = stp.reshape(nch, 64, 16).transpose(2, 0, 1).reshape(16, nch * 64)
        idxs.append(np.ascontiguousarray(np.tile(t16, (8, 1)).astype(np.int16)))
        dl = dl_stream[c * stream:(c + 1) * stream]
        iv = iv_stream[c * stream:(c + 1) * stream]
        valid = dl >= 0
        oh = np.zeros((128, nseg * 128), np.float32)
        oh[row_i[valid], seg_i[valid] * 128 + dl[valid]] = iv[valid]
        ohs.append(np.ascontiguousarray(oh.astype(bfloat16)))
    return idxs, ohs


def _fold_weights(p):
    f = lambda k: np.asarray(p[k], np.float32)
    W_ht, b_ht, W_lt, b_lt = f('W_ht'), f('b_ht'), f('W_lt'), f('b_lt')
    Ws_h, Wn_h, b_h = f('Ws_h'), f('Wn_h'), f('b_h')
    Ws_l, Wn_l, Wc_l, b_l = f('Ws_l'), f('Wn_l'), f('Wc_l'), f('b_l')
    Whr, bhr, Wlr, blr = f('Whr'), f('bhr'), f('Wlr'), f('blr')
    Wg, bg = f('Wg'), f('bg')
    Ah = W_ht @ (Ws_h + 0.2 * Whr)
    Al = W_lt @ (Ws_l + 0.2 * Wlr)
    Ag = 2.0 * Wg
    Bh, Bl, Cl = W_ht @ Wn_h, W_lt @ Wn_l, W_lt @ Wc_l
    c_h = b_ht @ (Ws_h + 0.2 * Whr) + b_h + 0.2 * bhr
    c_l = b_lt @ (Ws_l + 0.2 * Wlr) + b_l + 0.2 * blr
    c_r = bg
    u_h, u_l, u_c = b_ht @ Wn_h, b_lt @ Wn_l, b_lt @ Wc_l
    ones = np.ones((OD,), np.float32)

    cw = np.zeros((128, CWB_COLS), np.float32)
    rows_b = lambda b: slice(12 * b, 12 * b + 12)
    for b in range(B):
        cw[rows_b(b), CLB[f'w1_{b}']] = np.concatenate([Ah, Al], 1)
        cw[96, CLB[f'w1_{b}']] = np.concatenate([c_h, c_l])
        cw[rows_b(b), CLB[f'w2_{b}']] = np.concatenate([Bh, Bl], 1)
        cw[96, CLB[f'w2_{b}']] = np.concatenate([u_h, u_l])
        cw[rows_b(b), CLB[f'w3_{b}']] = Cl
        cw[96, CLB[f'w3_{b}']] = u_c
        cw[rows_b(b), CLB[f'wr_{b}']] = Ag
        cw[96, CLB[f'wr_{b}']] = c_r
        cw[rows_b(b), CLB[f'wsx_{b}']] = np.stack(
            [Ah @ ones, Al @ ones, Ag @ ones], 1)
        cw[96, CLB[f'wsx_{b}']] = [c_h.sum(), c_l.sum(), c_r.sum()]
        cw[rows_b(b), CLB[f'wsab_{b}']] = np.stack([Bh @ ones, Bl @ ones], 1)
        cw[96, CLB[f'wsab_{b}']] = [u_h.sum(), u_l.sum()]
        cw[rows_b(b), CLB[f'wsac_{b}']] = np.stack(
            [np.zeros(T, np.float32), Cl @ ones], 1)
        cw[96, CLB[f'wsac_{b}']] = [0.0, u_c.sum()]
    oneshl = np.zeros((128, 2), np.float32)
    oneshl[:64, 0] = 1.0
    oneshl[64:, 1] = 1.0
    cw[:, CLB['oneshl']] = oneshl
    cw[0:64, CLB['ones64']] = 1.0
    cw[:, CLB['wad']] = (f('Wa')[:, 0] - f('Wa')[:, 1])[:, None]
    cw[0, CLB['ehl']] = np.r_[np.ones(64, np.float32), np.zeros(64, np.float32)]
    cw[1, CLB['ehl']] = np.r_[np.zeros(64, np.float32), np.ones(64, np.float32)]
    cw[0, CLB['ecol0']] = np.r_[np.ones(64, np.float32),
                                np.zeros(64, np.float32)]
    cw[0, CLB['ecol1']] = np.r_[np.zeros(64, np.float32),
                                np.ones(64, np.float32)]
    cw[0, CLB['onesr']] = 1.0
    cw[:, CLB['ident2']] = np.vstack([np.eye(64, dtype=np.float32),
                                      np.eye(64, dtype=np.float32)])
    cwb = cw.astype(bfloat16)

    cwf = np.zeros((128, CWF_COLS), np.float32)
    ba = f('ba')
    cwf[0:3 * GRP, CLF['eps']] = EPS
    cwf[0:GRP, CLF['bad']] = float(ba[0] - ba[1])
    cwf[0:GRP, CLF['c13']] = 1.3
    cwf[:, CLF['gv']] = np.concatenate([f('g_hn'), f('g_ln')])[:, None]
    cwf[:, CLF['bv']] = np.concatenate([f('b_hn'), f('b_ln')])[:, None]
    cwf[0:64, CLF['gr']] = (0.1 * f('g_gn'))[:, None]
    cwf[0:64, CLF['br']] = (0.1 * f('b_gn'))[:, None]
    return cwb, cwf, float(ba[0] - ba[1])


# -------------------------------------------------------------- bass program
def _build_program(nseg, nch, ba_diff, repeat=1):
    import concourse.tile as tile
    from concourse import bacc, mybir

    f32 = mybir.dt.float32
    bf16 = mybir.dt.bfloat16
    i16 = mybir.dt.int16
    AF = mybir.ActivationFunctionType
    OP = mybir.AluOpType
    SEG_PER_BLK = nseg // NBLK

    nc = bacc.Bacc("TRN2", target_bir_lowering=False, debug=False,
                   enable_asserts=False, num_devices=NCORE)

    xT_d = nc.dram_tensor("xT", [NROWS, 128], bf16, kind="ExternalInput")
    rep_d = nc.dram_tensor("rep_tag", [1, 64 * repeat], f32,
                           kind="ExternalInput")
    xsb_d = nc.dram_tensor("xsb", [128, NPCP], bf16, kind="ExternalInput")
    cwb_d = nc.dram_tensor("cwb", [128, CWB_COLS], bf16, kind="ExternalInput")
    cwf_d = nc.dram_tensor("cwf", [128, CWF_COLS], f32, kind="ExternalInput")
    idx_d, oh_d = {}, {}
    for g in "bc":
        idx_d[g] = nc.dram_tensor(f"idx_{g}", [128, nch * 64], i16,
                                  kind="ExternalInput")
        oh_d[g] = nc.dram_tensor(f"oh_{g}", [128, nseg * 128], bf16,
                                 kind="ExternalInput")
    fo_d = nc.dram_tensor("fo", [B, OD, NPCP], f32, kind="ExternalOutput")
    ho_d = nc.dram_tensor("ho", [B, OD, NPCP], f32, kind="ExternalOutput")
    lo_d = nc.dram_tensor("lo", [B, OD, NPCP], f32, kind="ExternalOutput")

    def mmg(mms):
        """Emit matmuls as one PSUM accumulation group.
        mms: list of (out_ap, lhsT_ap, rhs_ap, tile_position)."""
        nmm = len(mms)
        for i, (out, lhsT, rhs, tp) in enumerate(mms):
            nc.tensor.matmul(out, lhsT, rhs, start=(i == 0),
                             stop=(i == nmm - 1), skip_group_check=True,
                             tile_position=tp)

    with tile.TileContext(nc) as tc:
        with (
            tc.tile_pool(name="const", bufs=1) as cpool,
        ):
            cwb = cpool.tile([128, CWB_COLS], bf16, tag="cwb")
            nc.sync.dma_start(cwb[:], cwb_d.ap())
            cwf = cpool.tile([128, CWF_COLS], f32, tag="cwf")
            nc.sync.dma_start(cwf[:], cwf_d.ap())

            def Wf(name, rows=128):
                return cwb[0:rows, CLB[name]]

            def Ff(name, rows=128):
                return cwf[0:rows, CLF[name]]

            rep_t = cpool.tile([1, 64], f32, tag="rep")
            nc.sync.dma_start(rep_t[:], rep_d.ap()[:, 0:64])
            xsb = cpool.tile([128, NPCP], bf16, tag="xsb")
            nc.sync.dma_start(xsb[:], xsb_d.ap())
            idx_t, aggT = {}, {}
            for g in "bc":
                idx_t[g] = cpool.tile([128, nch * 64], i16, tag=f"idx{g}",
                                      name=f"idx{g}")
                nc.sync.dma_start(idx_t[g][:], idx_d[g].ap())
                aggT[g] = cpool.tile([128, NPCP], bf16, tag=f"agg{g}",
                                     name=f"aggT{g}")
                nc.gpsimd.memset(aggT[g][:], 0.0)

            # ---- interleaved gather + dense, per chunk-group ----
            # Pair groups are chunk-major: dense work on columns [c0, c0+kl)
            # starts as soon as both graphs' aggT blocks covering those
            # columns are reduced, overlapping with later gathers.
            for _rep in range(repeat):
              with (
                  tc.tile_pool(name="gat", bufs=4) as gpool,
                  tc.tile_pool(name="ohp", bufs=6) as ohpool,
                  tc.tile_pool(name="aggps", bufs=2, space="PSUM") as aggps,
                  tc.tile_pool(name="mainps", bufs=3, space="PSUM") as mainps,
                  tc.tile_pool(name="statps", bufs=1, space="PSUM") as statps,
                  tc.tile_pool(name="ebc", bufs=2, space="PSUM") as ebcps,
                  tc.tile_pool(name="shl", bufs=GRP + 2) as shlpool,
                  tc.tile_pool(name="sr", bufs=GRP + 2) as srpool,
                  tc.tile_pool(name="hla", bufs=GRP + 2) as hlapool,
                  tc.tile_pool(name="sq", bufs=2) as sqpool,
                  tc.tile_pool(name="ssb", bufs=2) as ssbpool,
                  tc.tile_pool(name="stg", bufs=2) as stgpool,
                  tc.tile_pool(name="tmp", bufs=2) as tmppool,
                  tc.tile_pool(name="stat", bufs=1) as statpool,
                  tc.tile_pool(name="smax", bufs=1) as smaxpool,
              ):
               ps_blk = {g: None for g in "bc"}
               oh_blk = {g: None for g in "bc"}
               chunks_done = {g: 0 for g in "bc"}

               def emit_gather(g, upto):
                   for k in range(chunks_done[g], min(upto, nch)):
                       gt = gpool.tile([128, 8 * 128], bf16, tag="g")
                       gt3 = gt[:].rearrange("p (c e) -> p c e", e=128)
                       nc.gpsimd.dma_gather(
                           gt3, xT_d.ap(),
                           idx_t[g][:, k * 64:(k + 1) * 64],
                           num_idxs=1024, num_idxs_reg=1024, elem_size=128)
                       for c in range(8):
                           s = k * 8 + c
                           if s >= nseg:
                               break
                           r = s % SEG_PER_BLK
                           j = s // SEG_PER_BLK
                           if r == 0:
                               ps_blk[g] = aggps.tile([128, 128], f32,
                                                      tag="agg",
                                                      name=f"ps_{g}{j}")
                               ohb = ohpool.tile(
                                   [128, SEG_PER_BLK * 128], bf16, tag="oh",
                                   name=f"oh_{g}{j}")
                               oh_blk[g] = ohb
                               cs = slice(j * SEG_PER_BLK * 128,
                                          (j + 1) * SEG_PER_BLK * 128)
                               nc.sync.dma_start(ohb[:], oh_d[g].ap()[:, cs])
                           nc.tensor.matmul(
                               ps_blk[g][0:97, :], gt3[:, c, 0:97],
                               oh_blk[g][:, r * 128:(r + 1) * 128],
                               start=(r == 0), stop=(r == SEG_PER_BLK - 1),
                               skip_group_check=True, tile_position=(0, 0))
                           if r == SEG_PER_BLK - 1:
                               nc.scalar.activation(
                                   aggT[g][0:97, j * 128:(j + 1) * 128],
                                   ps_blk[g][0:97, :], AF.Copy)
                   chunks_done[g] = min(upto, nch)

               for grp, (c0g, klg) in enumerate(CHUNKS):
                  last_blk = (c0g + klg - 1) // 128
                  end_seg = (last_blk + 1) * SEG_PER_BLK
                  upto = -(-end_seg // 8)
                  if grp == len(CHUNKS) - 1:
                      upto = nch
                  for g in "bc":
                      emit_gather(g, upto)
                  gpairs = list(enumerate(
                      (b, c0g, klg) for b in range(B)))
                  st1 = statpool.tile([3 * GRP, 512], f32, tag="st1")
                  st2 = statpool.tile([3 * GRP, 512], f32, tag="st2")
                  sdt = smaxpool.tile([GRP, 512], f32, tag="sdt")
                  nc.gpsimd.memset(st1[:], 0.0)
                  nc.gpsimd.memset(st2[:], 1.0)
                  nc.gpsimd.memset(sdt[:], 0.0)
                  shl_t, sr_t, hla_t = {}, {}, {}

                  for q, (b, c0, kl) in gpairs:
                      xr = xsb[:, c0:c0 + kl]
                      ab = aggT['b'][:, c0:c0 + kl]
                      ac = aggT['c'][:, c0:c0 + kl]

                      phl = mainps.tile([128, 512], f32, tag="phl")
                      mmg([(phl[:, 0:kl], Wf(f'w1_{b}'), xr, (0, 0)),
                           (phl[:, 0:kl], Wf(f'w2_{b}'), ab, (0, 0)),
                           (phl[64:128, 0:kl], Wf(f'w3_{b}'), ac, (0, 64))])
                      pres = mainps.tile([64, 512], f32, tag="phl")
                      mmg([(pres[:, 0:kl], Wf(f'wr_{b}'), xr, (0, 0))])

                      sh = shlpool.tile([128, 512], bf16, tag="shl")
                      shl_t[q] = sh
                      nc.scalar.activation(sh[:, 0:kl], phl[:, 0:kl], AF.Copy)
                      sr = srpool.tile([64, 512], bf16, tag="sr",
                                       name=f"sr{q}")
                      sr_t[q] = sr
                      nc.scalar.activation(sr[0:64, 0:kl],
                                           pres[:, 0:kl], AF.Copy)
                      sq = sqpool.tile([128, 512], bf16, tag="sq")
                      nc.scalar.activation(sq[:, 0:kl], sh[:, 0:kl], AF.Square)
                      sqr = sqpool.tile([64, 512], bf16, tag="sqr")
                      nc.scalar.activation(sqr[:, 0:kl],
                                           sr[0:64, 0:kl], AF.Square)

                      # stats psum: sums@0:3, sumsq_hl@32:34, sumsq_r@64:65
                      S = statps.tile([65, 512], f32, tag="S")
                      mmg([(S[0:3, 0:kl], Wf(f'wsx_{b}'), xr, (0, 0)),
                           (S[0:2, 0:kl], Wf(f'wsab_{b}'), ab, (0, 0)),
                           (S[0:2, 0:kl], Wf(f'wsac_{b}'), ac, (0, 0))])
                      mmg([(S[32:34, 0:kl], Wf('oneshl'), sq[:, 0:kl],
                            (0, 32))])
                      mmg([(S[64:65, 0:kl], Wf('ones64', 64), sqr[:, 0:kl],
                            (0, 64))])
                      ssb = ssbpool.tile([65, 512], f32, tag="ssb")
                      nc.scalar.activation(ssb[0:3, 0:kl], S[0:3, 0:kl],
                                           AF.Copy)
                      nc.scalar.activation(ssb[32:34, 0:kl], S[32:34, 0:kl],
                                           AF.Copy)
                      nc.scalar.activation(ssb[64:65, 0:kl], S[64:65, 0:kl],
                                           AF.Copy)
                      nc.sync.dma_start(st1[3 * q:3 * q + 3, 0:kl],
                                        ssb[0:3, 0:kl])
                      nc.sync.dma_start(st2[3 * q:3 * q + 2, 0:kl],
                                        ssb[32:34, 0:kl])
                      nc.sync.dma_start(st2[3 * q + 2:3 * q + 3, 0:kl],
                                        ssb[64:65, 0:kl])

                  # ---- batched stats math (f32) ----
                  m_t = statpool.tile([3 * GRP, 512], f32, tag="m")
                  nc.vector.tensor_scalar_mul(m_t[:], st1[:], 1.0 / OD)
                  q_t = statpool.tile([3 * GRP, 512], f32, tag="q")
                  nc.vector.tensor_scalar_mul(q_t[:], st2[:], 1.0 / OD)
                  msq = statpool.tile([3 * GRP, 512], f32, tag="msq")
                  nc.vector.tensor_mul(msq[:], m_t[:], m_t[:])
                  var = statpool.tile([3 * GRP, 512], f32, tag="var")
                  nc.vector.tensor_sub(var[:], q_t[:], msq[:])
                  std = statpool.tile([3 * GRP, 512], f32, tag="std")
                  nc.scalar.activation(std[:], var[:], AF.Sqrt,
                                       bias=Ff('eps', 3 * GRP))
                  rstd = statpool.tile([3 * GRP, 512], f32, tag="rstd")
                  nc.vector.reciprocal(rstd[:], std[:])
                  mrstd = statpool.tile([3 * GRP, 512], f32, tag="mrstd")
                  nc.vector.tensor_mul(mrstd[:], m_t[:], rstd[:])
                  rstdb = statpool.tile([3 * GRP, 512], bf16, tag="rstdb")
                  nc.scalar.activation(rstdb[:], rstd[:], AF.Copy)
                  mrstdb = statpool.tile([3 * GRP, 512], bf16, tag="mrstdb")
                  nc.scalar.activation(mrstdb[:], mrstd[:], AF.Copy)

                  # ---- per-pair LN apply + activations + logit diff ----
                  for q, (b, c0, kl) in gpairs:
                      sh = shl_t[q]
                      rstg = stgpool.tile([2, 512], bf16, tag="rstg")
                      nc.sync.dma_start(rstg[:, 0:kl],
                                        rstdb[3 * q:3 * q + 2, 0:kl])
                      mstg = stgpool.tile([2, 512], bf16, tag="mstg")
                      nc.sync.dma_start(mstg[:, 0:kl],
                                        mrstdb[3 * q:3 * q + 2, 0:kl])
                      rbc = ebcps.tile([128, 512], f32, tag="ebc")
                      nc.tensor.matmul(rbc[:, 0:kl], Wf('ehl', 2),
                                       rstg[:, 0:kl],
                                       start=True, stop=True,
                                       skip_group_check=True,
                                       tile_position=(0, 0))
                      mbc = ebcps.tile([128, 512], f32, tag="ebc")
                      nc.tensor.matmul(mbc[:, 0:kl], Wf('ehl', 2),
                                       mstg[:, 0:kl],
                                       start=True, stop=True,
                                       skip_group_check=True,
                                       tile_position=(0, 0))
                      t1 = tmppool.tile([128, 512], bf16, tag="t1")
                      nc.vector.tensor_mul(t1[:, 0:kl], sh[:, 0:kl],
                                           rbc[:, 0:kl])
                      t2 = tmppool.tile([128, 512], bf16, tag="t2")
                      nc.vector.tensor_sub(t2[:, 0:kl], t1[:, 0:kl],
                                           mbc[:, 0:kl])
                      hla = hlapool.tile([128, 512], bf16, tag="hla")
                      hla_t[q] = hla
                      yh = tmppool.tile([64, 512], bf16, tag="yh")
                      nc.scalar.activation(yh[:, 0:kl], t2[0:64, 0:kl],
                                           AF.Identity, bias=Ff('bv')[0:64, :],
                                           scale=Ff('gv')[0:64, :])
                      nc.vector.scalar_tensor_tensor(
                          hla[0:64, 0:kl], yh[:, 0:kl], 0.1, yh[:, 0:kl],
                          OP.mult, OP.max)
                      nc.scalar.activation(hla[64:128, 0:kl], t2[64:128, 0:kl],
                                           AF.Gelu, bias=Ff('bv')[64:128, :],
                                           scale=Ff('gv')[64:128, :])
                      hout = tmppool.tile([128, 512], f32, tag="hout")
                      nc.scalar.activation(hout[:, 0:kl], hla[:, 0:kl],
                                           AF.Copy)
                      nc.sync.dma_start(ho_d.ap()[b, :, c0:c0 + kl],
                                        hout[0:64, 0:kl])
                      nc.sync.dma_start(lo_d.ap()[b, :, c0:c0 + kl],
                                        hout[64:128, 0:kl])
                      sd = statps.tile([1, 512], f32, tag="S")
                      nc.tensor.matmul(sd[:, 0:kl], Wf('wad'), hla[:, 0:kl],
                                       start=True, stop=True,
                                       skip_group_check=True,
                                       tile_position=(0, 0))
                      sdb = ssbpool.tile([1, 512], f32, tag="sdb")
                      nc.scalar.activation(sdb[:, 0:kl], sd[:, 0:kl], AF.Copy)
                      nc.sync.dma_start(sdt[q:q + 1, 0:kl], sdb[:, 0:kl])

                  # ---- batched 2-way softmax (f32 in, bf16 staged) ----
                  a0 = smaxpool.tile([GRP, 512], f32, tag="a0")
                  nc.scalar.activation(a0[:], sdt[:], AF.Sigmoid,
                                       bias=Ff('bad', GRP))
                  w0 = smaxpool.tile([GRP, 512], bf16, tag="w0")
                  nc.vector.tensor_scalar_add(w0[:], a0[:], 0.3)
                  w1_ = smaxpool.tile([GRP, 512], bf16, tag="w1_")
                  nc.scalar.activation(w1_[:], a0[:], AF.Identity,
                                       bias=Ff('c13', GRP), scale=-1.0)

                  # ---- per-pair fusion + residual + output ----
                  for q, (b, c0, kl) in gpairs:
                      hla = hla_t[q]
                      sr = sr_t[q]
                      w0s = stgpool.tile([1, 512], bf16, tag="w0s")
                      nc.sync.dma_start(w0s[:, 0:kl], w0[q:q + 1, 0:kl])
                      w1s = stgpool.tile([1, 512], bf16, tag="w1s")
                      nc.sync.dma_start(w1s[:, 0:kl], w1_[q:q + 1, 0:kl])
                      wbc = ebcps.tile([128, 512], f32, tag="ebc")
                      nc.tensor.matmul(wbc[:, 0:kl], Wf('ecol0', 1),
                                       w0s[:, 0:kl], start=True,
                                       stop=False, skip_group_check=True,
                                       tile_position=(0, 0))
                      nc.tensor.matmul(wbc[:, 0:kl], Wf('ecol1', 1),
                                       w1s[:, 0:kl], start=False,
                                       stop=True, skip_group_check=True,
                                       tile_position=(0, 0))
                      f1 = tmppool.tile([128, 512], bf16, tag="f1")
                      nc.vector.tensor_mul(f1[:, 0:kl], hla[:, 0:kl],
                                           wbc[:, 0:kl])
                      rrs = stgpool.tile([1, 512], bf16, tag="rrs")
                      nc.sync.dma_start(rrs[:, 0:kl],
                                        rstdb[3 * q + 2:3 * q + 3, 0:kl])
                      rms = stgpool.tile([1, 512], bf16, tag="rms")
                      nc.sync.dma_start(rms[:, 0:kl],
                                        mrstdb[3 * q + 2:3 * q + 3, 0:kl])
                      rr = ebcps.tile([64, 512], f32, tag="ebc")
                      nc.tensor.matmul(rr[:, 0:kl], Wf('onesr', 1),
                                       rrs[:, 0:kl],
                                       start=True, stop=True,
                                       skip_group_check=True,
                                       tile_position=(0, 0))
                      rm = ebcps.tile([64, 512], f32, tag="ebc")
                      nc.tensor.matmul(rm[:, 0:kl], Wf('onesr', 1),
                                       rms[:, 0:kl],
                                       start=True, stop=True,
                                       skip_group_check=True,
                                       tile_position=(0, 0))
                      u1 = tmppool.tile([64, 512], bf16, tag="u1")
                      nc.vector.tensor_mul(u1[:, 0:kl], sr[0:64, 0:kl],
                                           rr[:, 0:kl])
                      u2 = tmppool.tile([64, 512], bf16, tag="u2")
                      nc.vector.tensor_sub(u2[:, 0:kl], u1[:, 0:kl],
                                           rm[:, 0:kl])
                      resa = tmppool.tile([64, 512], bf16, tag="resa")
                      nc.scalar.activation(resa[:, 0:kl], u2[:, 0:kl],
                                           AF.Identity, bias=Ff('br')[0:64, :],
                                           scale=Ff('gr')[0:64, :])
                      f2 = ebcps.tile([64, 512], f32, tag="ebc")
                      nc.tensor.matmul(f2[:, 0:kl], Wf('ident2'), f1[:, 0:kl],
                                       start=True, stop=True,
                                       skip_group_check=True,
                                       tile_position=(0, 0))
                      f3 = tmppool.tile([64, 512], f32, tag="f3")
                      nc.vector.tensor_add(f3[:, 0:kl], f2[:, 0:kl],
                                           resa[:, 0:kl])
                      nc.sync.dma_start(fo_d.ap()[b, :, c0:c0 + kl],
                                        f3[:, 0:kl])
    nc.finalize()
    return nc


# ------------------------------------------------------------------- runner
class _SpmdRunner:
    def __init__(self, nc, n_cores=NCORE):
        import jax
        from jax.sharding import Mesh, PartitionSpec
        from jax.experimental.shard_map import shard_map
        from concourse import mybir
        from concourse.bass2jax import (_bass_exec_p, install_neuronx_cc_hook,
                                        partition_id_tensor)
        install_neuronx_cc_hook()
        self.jax = jax
        self.n_cores = n_cores
        partition_name = (nc.partition_id_tensor.name
                          if nc.partition_id_tensor else None)
        in_names, out_names, out_avals, zero_outs = [], [], [], []
        for alloc in nc.m.functions[0].allocations:
            if not isinstance(alloc, mybir.MemoryLocationSet):
                continue
            name = alloc.memorylocations[0].name
            if alloc.kind == "ExternalInput":
                if name != partition_name:
                    in_names.append(name)
            elif alloc.kind == "ExternalOutput":
                out_names.append(name)
                shape = tuple(alloc.tensor_shape)
                dtype = mybir.dt.np(alloc.dtype)
                out_avals.append(jax.core.ShapedArray(shape, dtype))
                zero_outs.append(np.zeros(shape, dtype))
        self.in_names, self.out_names = in_names, out_names
        self.out_avals = out_avals
        n_params, n_outs = len(in_names), len(out_avals)
        all_in = list(in_names) + list(out_names)
        if partition_name is not None:
            all_in.append(partition_name)

        def _body(*args):
            operands = list(args)
            if partition_name is not None:
                operands.append(partition_id_tensor())
            outs = _bass_exec_p.bind(
                *operands, out_avals=tuple(out_avals),
                in_names=tuple(all_in), out_names=tuple(out_names),
                lowering_input_output_aliases=(),
                sim_require_finite=True, sim_require_nnan=True, nc=nc)
            return tuple(outs)

        devices = jax.devices()[:n_cores]
        mesh = Mesh(np.asarray(devices), ("core",))
        self.mesh = mesh
        in_specs = (PartitionSpec("core"),) * (n_params + n_outs)
        out_specs = (PartitionSpec("core"),) * n_outs
        self.fn = jax.jit(
            shard_map(_body, mesh=mesh, in_specs=in_specs,
                      out_specs=out_specs, check_rep=False),
            keep_unused=True)
        self._concat_zeros = [
            np.zeros((n_cores * z.shape[0], *z.shape[1:]), z.dtype)
            for z in zero_outs]

    def prepare(self, in_maps):
        from jax.sharding import NamedSharding, PartitionSpec
        n = self.n_cores
        per_core = [[np.ascontiguousarray(m[name]) for name in self.in_names]
                    for m in in_maps]
        concat_in = [np.concatenate([per_core[c][i] for c in range(n)], axis=0)
                     for i in range(len(self.in_names))]
        args = concat_in + self._concat_zeros
        # Pre-shard along dim 0 so each run() call dispatches the kernel
        # directly instead of inserting per-call resharding copies.
        sh = NamedSharding(self.mesh, PartitionSpec("core"))
        return [self.jax.device_put(a, sh) for a in args]

    def run(self, args):
        outs = self.fn(*args)
        self.jax.block_until_ready(outs)
        return outs

    def split_outs(self, outs):
        res = []
        for c in range(self.n_cores):
            d = {}
            for i, name in enumerate(self.out_names):
                d[name] = np.asarray(outs[i]).reshape(
                    self.n_cores, *self.out_avals[i].shape)[c]
            res.append(d)
        return res


# -------------------------------------------------------------------- entry
def _get(inputs):
    gb = _prep_graph(inputs['edge_index'])
    gc = _prep_graph(inputs['causal_edge_index'])
    b_pad = max(128, -(-int(max(gb['counts'].max(), gc['counts'].max()))
                     // 128) * 128)
    stream = NBLK * b_pad
    nseg = stream // 128
    nch = -(-stream // 1024)
    cwb, cwf, ba_diff = _fold_weights(inputs)
    key = (b_pad, nseg, nch, round(ba_diff, 9), REPEAT)
    if key not in _cache:
        nc = _build_program(nseg, nch, ba_diff, REPEAT)
        _cache[key] = _SpmdRunner(nc)
    return _cache[key], gb, gc, b_pad, nseg, nch, cwb, cwf


def make_in_maps(inputs):
    runner, gb, gc, b_pad, nseg, nch, cwb, cwf = _get(inputs)
    x = np.asarray(inputs['x'], np.float32)
    xflat = np.zeros((128, N), np.float32)
    xflat[0:96] = x.reshape(96, N)
    xflat[96] = 1.0
    xT = np.zeros((NROWS, 128), np.float32)
    xT[:N, 0:97] = xflat[0:97].T
    xT16 = xT.astype(bfloat16)
    idx_b, oh_b = _build_streams(gb, b_pad, nseg, nch)
    idx_c, oh_c = _build_streams(gc, b_pad, nseg, nch)
    in_maps = []
    for c in range(NCORE):
        xs = np.zeros((128, NPCP), np.float32)
        xs[:, 0:NPC] = xflat[:, c * NPC:(c + 1) * NPC]
        in_maps.append({
            'xT': xT16, 'xsb': xs.astype(bfloat16), 'cwb': cwb, 'cwf': cwf,
            'rep_tag': np.zeros((1, 64 * REPEAT), np.float32),
            'idx_b': idx_b[c], 'oh_b': oh_b[c],
            'idx_c': idx_c[c], 'oh_c': oh_c[c],
        })
    return runner, in_maps


def kernel(**inputs):
    runner, in_maps = make_in_maps(inputs)
    args = runner.prepare(in_maps)
    outs = runner.run(args)
    res = runner.split_outs(outs)
    fused = np.empty((B, OD, N), np.float32)
    high = np.empty((B, OD, N), np.float32)
    low = np.empty((B, OD, N), np.float32)
    for c in range(NCORE):
        sl = slice(c * NPC, (c + 1) * NPC)
        fused[:, :, sl] = res[c]['fo'][:, :, 0:NPC]
        high[:, :, sl] = res[c]['ho'][:, :, 0:NPC]
        low[:, :, sl] = res[c]['lo'][:, :, 0:NPC]
    return fused, high, low


# revision 12
# speedup vs baseline: 176.8084x; 1.0239x over previous
"""Trainium2 Bass kernel for nn_DWTEnhancedSTGCN (B=8, T=12, N=10000, E=160000).

Strategy (N-sharded over 8 NeuronCores):
  - Each core owns 1250 dst-nodes for ALL 8 batch elements; edges are shared
    across the batch, so each edge's 96 batch-features (8b x 12t) are gathered
    ONCE per core via dma_gather (256B bf16 rows from an xT scratch in HBM).
  - Aggregation = mean over in-edges. Linearity lets us aggregate the 12-dim
    x features instead of 64-dim hidden features; mean normalization (invdeg)
    is folded into host-precomputed one-hot segment matrices
       onehot[e, j] = (j == dstloc[e]) * invdeg[dst[e]]   (bf16, uploaded),
    then aggT_block += G_chunk.T @ onehot on the PE.
  - Dense phase in [feature, node] layout with K=128 zero-padded weight
    blocks per batch (biases ride on the all-ones row 96). All matmuls and
    elementwise work run in bf16 (f32 PSUM accumulation); LayerNorm stats
    math stays f32. Activations on ACT; fusion on DVE.
Host does only: sharding/reshapes, integer index-stream building, and
parameter-only weight folding (incl. the graph-structure one-hot streams).
All FP math on x runs on device.
"""
import sys
import numpy as np
from ml_dtypes import bfloat16

sys.path.insert(0, '/opt/trn_rl_repo')

B, T, N = 8, 12, 10000
OD = 64
NCORE = 8
NPC = N // NCORE          # 1250 nodes per core
NPCP = 1280               # padded local node count (10 blocks of 128)
NBLK = NPCP // 128
EPS = 1e-5
PADROW = N                # all-zero row in xT used by padding gather idxs
NROWS = N + 16
CHUNKS = [(0, 512), (512, 512), (1024, 256)]
NPAIR = B * len(CHUNKS)   # 24
GRP = 8                   # pairs per stats/softmax group
NGRP = NPAIR // GRP
REPEAT = 1                # in-kernel repetition (timing mode)

# column layout of the packed bf16 constant tile cwb [128, CWB_COLS]
CLB = {}
_cb = 0
def _clb(name, cols):
    global _cb
    CLB[name] = slice(_cb, _cb + cols)
    _cb += cols
for _b in range(B):
    _clb(f'w1_{_b}', 128)   # [A_h|A_l] at rows 12b..12b+11, biases at row 96
    _clb(f'w2_{_b}', 128)   # [B_h|B_l] + u at row 96
    _clb(f'w3_{_b}', 64)    # C_l + u_c at row 96
    _clb(f'wr_{_b}', 64)    # Ag + c_r at row 96
    _clb(f'wsx_{_b}', 3)    # col sums of (high, low, res) x-parts (+bias sums)
    _clb(f'wsab_{_b}', 2)
    _clb(f'wsac_{_b}', 2)
_clb('oneshl', 2)
_clb('ones64', 1)
_clb('wad', 1)
_clb('ehl', 128)
_clb('ecol0', 128)
_clb('ecol1', 128)
_clb('onesr', 64)
_clb('ident2', 64)
CWB_COLS = -(-_cb // 64) * 64

# f32 constant tile cwf [128, CWF_COLS] (ACT bias columns for f32 math)
CLF = {}
_cf = 0
def _clf(name, cols):
    global _cf
    CLF[name] = slice(_cf, _cf + cols)
    _cf += cols
_clf('eps', 1)
_clf('bad', 1)
_clf('c13', 1)
_clf('gv', 1)
_clf('bv', 1)
_clf('gr', 1)
_clf('br', 1)
CWF_COLS = 64

_cache = {}


# ----------------------------------------------------------------- host prep
def _prep_graph(edge_index):
    src = np.asarray(edge_index[0]).astype(np.int64).ravel()
    dst = np.asarray(edge_index[1]).astype(np.int64).ravel()
    deg = np.bincount(dst, minlength=N)
    invdeg = (1.0 / np.maximum(deg, 1)).astype(np.float32)
    order = np.argsort(dst, kind='stable')
    s_s, d_s = src[order], dst[order]
    core = d_s // NPC
    local = d_s - core * NPC
    blk = local >> 7
    dstloc = local & 127
    binid = core * NBLK + blk
    counts = np.bincount(binid, minlength=NCORE * NBLK)
    return dict(s=s_s, d=d_s, core=core, binid=binid, dstloc=dstloc,
                blk=blk, counts=counts, invdeg=invdeg)


def _build_streams(g, b_pad, nseg, nch):
    stream = NBLK * b_pad
    starts = np.zeros(NCORE * NBLK, np.int64)
    np.cumsum(g['counts'][:-1], out=starts[1:])
    rank = np.arange(len(g['s'])) - starts[g['binid']]
    pos = g['core'] * stream + g['blk'] * b_pad + rank
    src_stream = np.full(NCORE * stream, PADROW, np.int64)
    dl_stream = np.full(NCORE * stream, -1, np.int64)
    iv_stream = np.zeros(NCORE * stream, np.float32)
    src_stream[pos] = g['s']
    dl_stream[pos] = g['dstloc']
    iv_stream[pos] = g['invdeg'][g['d']]
    seg_i = (np.arange(stream) // 128).astype(np.int64)
    row_i = (np.arange(stream) % 128).astype(np.int64)
    idxs, ohs = [], []
    for c in range(NCORE):
        st = src_stream[c * stream:(c + 1) * stream]
        stp = np.full(nch * 1024, PADROW, np.int64)
        stp[:stream] = st
        t16 = stp.reshape(nch, 64, 16).transpose(2, 0, 1).reshape(16, nch * 64)
        idxs.append(np.ascontiguousarray(np.tile(t16, (8, 1)).astype(np.int16)))
        dl = dl_stream[c * stream:(c + 1) * stream]
        iv = iv_stream[c * stream:(c + 1) * stream]
        valid = dl >= 0
        oh = np.zeros((128, nseg * 128), np.float32)
        oh[row_i[valid], seg_i[valid] * 128 + dl[valid]] = iv[valid]
        ohs.append(np.ascontiguousarray(oh.astype(bfloat16)))
    return idxs, ohs


def _fold_weights(p):
    f = lambda k: np.asarray(p[k], np.float32)
    W_ht, b_ht, W_lt, b_lt = f('W_ht'), f('b_ht'), f('W_lt'), f('b_lt')
    Ws_h, Wn_h, b_h = f('Ws_h'), f('Wn_h'), f('b_h')
    Ws_l, Wn_l, Wc_l, b_l = f('Ws_l'), f('Wn_l'), f('Wc_l'), f('b_l')
    Whr, bhr, Wlr, blr = f('Whr'), f('bhr'), f('Wlr'), f('blr')
    Wg, bg = f('Wg'), f('bg')
    Ah = W_ht @ (Ws_h + 0.2 * Whr)
    Al = W_lt @ (Ws_l + 0.2 * Wlr)
    Ag = 2.0 * Wg
    Bh, Bl, Cl = W_ht @ Wn_h, W_lt @ Wn_l, W_lt @ Wc_l
    c_h = b_ht @ (Ws_h + 0.2 * Whr) + b_h + 0.2 * bhr
    c_l = b_lt @ (Ws_l + 0.2 * Wlr) + b_l + 0.2 * blr
    c_r = bg
    u_h, u_l, u_c = b_ht @ Wn_h, b_lt @ Wn_l, b_lt @ Wc_l
    ones = np.ones((OD,), np.float32)

    cw = np.zeros((128, CWB_COLS), np.float32)
    rows_b = lambda b: slice(12 * b, 12 * b + 12)
    for b in range(B):
        cw[rows_b(b), CLB[f'w1_{b}']] = np.concatenate([Ah, Al], 1)
        cw[96, CLB[f'w1_{b}']] = np.concatenate([c_h, c_l])
        cw[rows_b(b), CLB[f'w2_{b}']] = np.concatenate([Bh, Bl], 1)
        cw[96, CLB[f'w2_{b}']] = np.concatenate([u_h, u_l])
        cw[rows_b(b), CLB[f'w3_{b}']] = Cl
        cw[96, CLB[f'w3_{b}']] = u_c
        cw[rows_b(b), CLB[f'wr_{b}']] = Ag
        cw[96, CLB[f'wr_{b}']] = c_r
        cw[rows_b(b), CLB[f'wsx_{b}']] = np.stack(
            [Ah @ ones, Al @ ones, Ag @ ones], 1)
        cw[96, CLB[f'wsx_{b}']] = [c_h.sum(), c_l.sum(), c_r.sum()]
        cw[rows_b(b), CLB[f'wsab_{b}']] = np.stack([Bh @ ones, Bl @ ones], 1)
        cw[96, CLB[f'wsab_{b}']] = [u_h.sum(), u_l.sum()]
        cw[rows_b(b), CLB[f'wsac_{b}']] = np.stack(
            [np.zeros(T, np.float32), Cl @ ones], 1)
        cw[96, CLB[f'wsac_{b}']] = [0.0, u_c.sum()]
    oneshl = np.zeros((128, 2), np.float32)
    oneshl[:64, 0] = 1.0
    oneshl[64:, 1] = 1.0
    cw[:, CLB['oneshl']] = oneshl
    cw[0:64, CLB['ones64']] = 1.0
    cw[:, CLB['wad']] = (f('Wa')[:, 0] - f('Wa')[:, 1])[:, None]
    cw[0, CLB['ehl']] = np.r_[np.ones(64, np.float32), np.zeros(64, np.float32)]
    cw[1, CLB['ehl']] = np.r_[np.zeros(64, np.float32), np.ones(64, np.float32)]
    cw[0, CLB['ecol0']] = np.r_[np.ones(64, np.float32),
                                np.zeros(64, np.float32)]
    cw[0, CLB['ecol1']] = np.r_[np.zeros(64, np.float32),
                                np.ones(64, np.float32)]
    cw[0, CLB['onesr']] = 1.0
    cw[:, CLB['ident2']] = np.vstack([np.eye(64, dtype=np.float32),
                                      np.eye(64, dtype=np.float32)])
    cwb = cw.astype(bfloat16)

    cwf = np.zeros((128, CWF_COLS), np.float32)
    ba = f('ba')
    cwf[0:3 * GRP, CLF['eps']] = EPS
    cwf[0:GRP, CLF['bad']] = float(ba[0] - ba[1])
    cwf[0:GRP, CLF['c13']] = 1.3
    cwf[:, CLF['gv']] = np.concatenate([f('g_hn'), f('g_ln')])[:, None]
    cwf[:, CLF['bv']] = np.concatenate([f('b_hn'), f('b_ln')])[:, None]
    cwf[0:64, CLF['gr']] = (0.1 * f('g_gn'))[:, None]
    cwf[0:64, CLF['br']] = (0.1 * f('b_gn'))[:, None]
    return cwb, cwf, float(ba[0] - ba[1])


# -------------------------------------------------------------- bass program
def _build_program(nseg, nch, ba_diff, repeat=1):
    import concourse.tile as tile
    from concourse import bacc, mybir

    f32 = mybir.dt.float32
    bf16 = mybir.dt.bfloat16
    i16 = mybir.dt.int16
    AF = mybir.ActivationFunctionType
    OP = mybir.AluOpType
    SEG_PER_BLK = nseg // NBLK

    nc = bacc.Bacc("TRN2", target_bir_lowering=False, debug=False,
                   enable_asserts=False, num_devices=NCORE)

    xT_d = nc.dram_tensor("xT", [NROWS, 128], bf16, kind="ExternalInput")
    rep_d = nc.dram_tensor("rep_tag", [1, 64 * repeat], f32,
                           kind="ExternalInput")
    xsb_d = nc.dram_tensor("xsb", [128, NPCP], bf16, kind="ExternalInput")
    cwb_d = nc.dram_tensor("cwb", [128, CWB_COLS], bf16, kind="ExternalInput")
    cwf_d = nc.dram_tensor("cwf", [128, CWF_COLS], f32, kind="ExternalInput")
    idx_d, oh_d = {}, {}
    for g in "bc":
        idx_d[g] = nc.dram_tensor(f"idx_{g}", [128, nch * 64], i16,
                                  kind="ExternalInput")
        oh_d[g] = nc.dram_tensor(f"oh_{g}", [128, nseg * 128], bf16,
                                 kind="ExternalInput")
    fo_d = nc.dram_tensor("fo", [B, OD, NPCP], bf16,
                      kind="ExternalOutput")
    ho_d = nc.dram_tensor("ho", [B, OD, NPCP], bf16,
                          kind="ExternalOutput")
    lo_d = nc.dram_tensor("lo", [B, OD, NPCP], bf16,
                          kind="ExternalOutput")

    def mmg(mms):
        """Emit matmuls as one PSUM accumulation group.
        mms: list of (out_ap, lhsT_ap, rhs_ap, tile_position)."""
        nmm = len(mms)
        for i, (out, lhsT, rhs, tp) in enumerate(mms):
            nc.tensor.matmul(out, lhsT, rhs, start=(i == 0),
                             stop=(i == nmm - 1), skip_group_check=True,
                             tile_position=tp)

    with tile.TileContext(nc) as tc:
        with (
            tc.tile_pool(name="const", bufs=1) as cpool,
        ):
            cwb = cpool.tile([128, CWB_COLS], bf16, tag="cwb")
            nc.sync.dma_start(cwb[:], cwb_d.ap())
            cwf = cpool.tile([128, CWF_COLS], f32, tag="cwf")
            nc.sync.dma_start(cwf[:], cwf_d.ap())

            def Wf(name, rows=128):
                return cwb[0:rows, CLB[name]]

            def Ff(name, rows=128):
                return cwf[0:rows, CLF[name]]

            rep_t = cpool.tile([1, 64], f32, tag="rep")
            nc.sync.dma_start(rep_t[:], rep_d.ap()[:, 0:64])
            xsb = cpool.tile([128, NPCP], bf16, tag="xsb")
            nc.sync.dma_start(xsb[:], xsb_d.ap())
            idx_t, aggT = {}, {}
            for g in "bc":
                idx_t[g] = cpool.tile([128, nch * 64], i16, tag=f"idx{g}",
                                      name=f"idx{g}")
                nc.sync.dma_start(idx_t[g][:], idx_d[g].ap())
                aggT[g] = cpool.tile([128, NPCP], bf16, tag=f"agg{g}",
                                     name=f"aggT{g}")
                nc.gpsimd.memset(aggT[g][:], 0.0)

            # ---- interleaved gather + dense, per chunk-group ----
            # Pair groups are chunk-major: dense work on columns [c0, c0+kl)
            # starts as soon as both graphs' aggT blocks covering those
            # columns are reduced, overlapping with later gathers.
            for _rep in range(repeat):
              with (
                  tc.tile_pool(name="gat", bufs=4) as gpool,
                  tc.tile_pool(name="ohp", bufs=6) as ohpool,
                  tc.tile_pool(name="aggps", bufs=2, space="PSUM") as aggps,
                  tc.tile_pool(name="mainps", bufs=2, space="PSUM") as mainps,
                  tc.tile_pool(name="statps", bufs=2, space="PSUM") as statps,
                  tc.tile_pool(name="ebc", bufs=2, space="PSUM") as ebcps,
                  tc.tile_pool(name="shl", bufs=GRP + 2) as shlpool,
                  tc.tile_pool(name="sr", bufs=GRP + 2) as srpool,
                  tc.tile_pool(name="hla", bufs=GRP + 2) as hlapool,
                  tc.tile_pool(name="sq", bufs=2) as sqpool,
                  tc.tile_pool(name="ssb", bufs=2) as ssbpool,
                  tc.tile_pool(name="stg", bufs=2) as stgpool,
                  tc.tile_pool(name="tmp", bufs=2) as tmppool,
                  tc.tile_pool(name="stat", bufs=2) as statpool,
                  tc.tile_pool(name="smax", bufs=2) as smaxpool,
              ):
               ps_blk = {g: None for g in "bc"}
               oh_blk = {g: None for g in "bc"}
               chunks_done = {g: 0 for g in "bc"}

               def emit_gather(g, upto):
                   for k in range(chunks_done[g], min(upto, nch)):
                       gt = gpool.tile([128, 8 * 128], bf16, tag="g")
                       gt3 = gt[:].rearrange("p (c e) -> p c e", e=128)
                       nc.gpsimd.dma_gather(
                           gt3, xT_d.ap(),
                           idx_t[g][:, k * 64:(k + 1) * 64],
                           num_idxs=1024, num_idxs_reg=1024, elem_size=128)
                       for c in range(8):
                           s = k * 8 + c
                           if s >= nseg:
                               break
                           r = s % SEG_PER_BLK
                           j = s // SEG_PER_BLK
                           if r == 0:
                               ps_blk[g] = aggps.tile([128, 128], f32,
                                                      tag="agg",
                                                      name=f"ps_{g}{j}")
                               ohb = ohpool.tile(
                                   [128, SEG_PER_BLK * 128], bf16, tag="oh",
                                   name=f"oh_{g}{j}")
                               oh_blk[g] = ohb
                               cs = slice(j * SEG_PER_BLK * 128,
                                          (j + 1) * SEG_PER_BLK * 128)
                               nc.sync.dma_start(ohb[:], oh_d[g].ap()[:, cs])
                           nc.tensor.matmul(
                               ps_blk[g][0:97, :], gt3[:, c, 0:97],
                               oh_blk[g][:, r * 128:(r + 1) * 128],
                               start=(r == 0), stop=(r == SEG_PER_BLK - 1),
                               skip_group_check=True, tile_position=(0, 0))
                           if r == SEG_PER_BLK - 1:
                               nc.scalar.activation(
                                   aggT[g][0:97, j * 128:(j + 1) * 128],
                                   ps_blk[g][0:97, :], AF.Copy)
                   chunks_done[g] = min(upto, nch)

               for grp, (c0g, klg) in enumerate(CHUNKS):
                  last_blk = (c0g + klg - 1) // 128
                  end_seg = (last_blk + 1) * SEG_PER_BLK
                  upto = -(-end_seg // 8)
                  if grp == len(CHUNKS) - 1:
                      upto = nch
                  for g in "bc":
                      emit_gather(g, upto)
                  gpairs = list(enumerate(
                      (b, c0g, klg) for b in range(B)))
                  st1 = statpool.tile([3 * GRP, 512], f32, tag="st1")
                  st2 = statpool.tile([3 * GRP, 512], f32, tag="st2")
                  sdt = smaxpool.tile([GRP, 512], f32, tag="sdt")
                  nc.gpsimd.memset(st1[:], 0.0)
                  nc.gpsimd.memset(st2[:], 1.0)
                  nc.gpsimd.memset(sdt[:], 0.0)
                  shl_t, sr_t, hla_t = {}, {}, {}

                  for q, (b, c0, kl) in gpairs:
                      xr = xsb[:, c0:c0 + kl]
                      ab = aggT['b'][:, c0:c0 + kl]
                      ac = aggT['c'][:, c0:c0 + kl]

                      phl = mainps.tile([128, 512], f32, tag="phl")
                      mmg([(phl[:, 0:kl], Wf(f'w1_{b}'), xr, (0, 0)),
                           (phl[:, 0:kl], Wf(f'w2_{b}'), ab, (0, 0)),
                           (phl[64:128, 0:kl], Wf(f'w3_{b}'), ac, (0, 64))])
                      pres = mainps.tile([64, 512], f32, tag="phl")
                      mmg([(pres[:, 0:kl], Wf(f'wr_{b}'), xr, (0, 0))])

                      sh = shlpool.tile([128, 512], bf16, tag="shl")
                      shl_t[q] = sh
                      nc.scalar.activation(sh[:, 0:kl], phl[:, 0:kl], AF.Copy)
                      sr = srpool.tile([64, 512], bf16, tag="sr",
                                       name=f"sr{q}")
                      sr_t[q] = sr
                      nc.scalar.activation(sr[0:64, 0:kl],
                                           pres[:, 0:kl], AF.Copy)
                      sq = sqpool.tile([128, 512], bf16, tag="sq")
                      nc.vector.tensor_mul(sq[:, 0:kl], sh[:, 0:kl],
                                           sh[:, 0:kl])
                      sqr = sqpool.tile([64, 512], bf16, tag="sqr")
                      nc.vector.tensor_mul(sqr[:, 0:kl], sr[0:64, 0:kl],
                                           sr[0:64, 0:kl])

                      # stats psum: sums@0:3, sumsq_hl@32:34, sumsq_r@64:65
                      S = statps.tile([65, 512], f32, tag="S")
                      mmg([(S[0:3, 0:kl], Wf(f'wsx_{b}'), xr, (0, 0)),
                           (S[0:2, 0:kl], Wf(f'wsab_{b}'), ab, (0, 0)),
                           (S[0:2, 0:kl], Wf(f'wsac_{b}'), ac, (0, 0))])
                      mmg([(S[32:34, 0:kl], Wf('oneshl'), sq[:, 0:kl],
                            (0, 32))])
                      mmg([(S[64:65, 0:kl], Wf('ones64', 64), sqr[:, 0:kl],
                            (0, 64))])
                      ssb = ssbpool.tile([65, 512], f32, tag="ssb")
                      nc.scalar.activation(ssb[0:3, 0:kl], S[0:3, 0:kl],
                                           AF.Copy)
                      nc.scalar.activation(ssb[32:34, 0:kl], S[32:34, 0:kl],
                                           AF.Copy)
                      nc.scalar.activation(ssb[64:65, 0:kl], S[64:65, 0:kl],
                                           AF.Copy)
                      nc.sync.dma_start(st1[3 * q:3 * q + 3, 0:kl],
                                        ssb[0:3, 0:kl])
                      nc.sync.dma_start(st2[3 * q:3 * q + 2, 0:kl],
                                        ssb[32:34, 0:kl])
                      nc.sync.dma_start(st2[3 * q + 2:3 * q + 3, 0:kl],
                                        ssb[64:65, 0:kl])

                  # ---- batched stats math (f32) ----
                  m_t = statpool.tile([3 * GRP, 512], f32, tag="m")
                  nc.vector.tensor_scalar_mul(m_t[:], st1[:], 1.0 / OD)
                  q_t = statpool.tile([3 * GRP, 512], f32, tag="q")
                  nc.vector.tensor_scalar_mul(q_t[:], st2[:], 1.0 / OD)
                  msq = statpool.tile([3 * GRP, 512], f32, tag="msq")
                  nc.vector.tensor_mul(msq[:], m_t[:], m_t[:])
                  var = statpool.tile([3 * GRP, 512], f32, tag="var")
                  nc.vector.tensor_sub(var[:], q_t[:], msq[:])
                  std = statpool.tile([3 * GRP, 512], f32, tag="std")
                  nc.scalar.activation(std[:], var[:], AF.Sqrt,
                                       bias=Ff('eps', 3 * GRP))
                  rstd = statpool.tile([3 * GRP, 512], f32, tag="rstd")
                  nc.vector.reciprocal(rstd[:], std[:])
                  mrstd = statpool.tile([3 * GRP, 512], f32, tag="mrstd")
                  nc.vector.tensor_mul(mrstd[:], m_t[:], rstd[:])
                  rstdb = statpool.tile([3 * GRP, 512], bf16, tag="rstdb")
                  nc.scalar.activation(rstdb[:], rstd[:], AF.Copy)
                  mrstdb = statpool.tile([3 * GRP, 512], bf16, tag="mrstdb")
                  nc.scalar.activation(mrstdb[:], mrstd[:], AF.Copy)

                  # ---- per-pair LN apply + activations + logit diff ----
                  for q, (b, c0, kl) in gpairs:
                      sh = shl_t[q]
                      rstg = stgpool.tile([2, 512], bf16, tag="rstg")
                      nc.sync.dma_start(rstg[:, 0:kl],
                                        rstdb[3 * q:3 * q + 2, 0:kl])
                      mstg = stgpool.tile([2, 512], bf16, tag="mstg")
                      nc.sync.dma_start(mstg[:, 0:kl],
                                        mrstdb[3 * q:3 * q + 2, 0:kl])
                      rbc = ebcps.tile([128, 512], f32, tag="ebc")
                      nc.tensor.matmul(rbc[:, 0:kl], Wf('ehl', 2),
                                       rstg[:, 0:kl],
                                       start=True, stop=True,
                                       skip_group_check=True,
                                       tile_position=(0, 0))
                      mbc = ebcps.tile([128, 512], f32, tag="ebc")
                      nc.tensor.matmul(mbc[:, 0:kl], Wf('ehl', 2),
                                       mstg[:, 0:kl],
                                       start=True, stop=True,
                                       skip_group_check=True,
                                       tile_position=(0, 0))
                      t1 = tmppool.tile([128, 512], bf16, tag="t1")
                      nc.vector.tensor_mul(t1[:, 0:kl], sh[:, 0:kl],
                                           rbc[:, 0:kl])
                      t2 = tmppool.tile([128, 512], bf16, tag="t2")
                      nc.vector.tensor_sub(t2[:, 0:kl], t1[:, 0:kl],
                                           mbc[:, 0:kl])
                      hla = hlapool.tile([128, 512], bf16, tag="hla")
                      hla_t[q] = hla
                      yh = tmppool.tile([64, 512], bf16, tag="yh")
                      nc.scalar.activation(yh[:, 0:kl], t2[0:64, 0:kl],
                                           AF.Identity, bias=Ff('bv')[0:64, :],
                                           scale=Ff('gv')[0:64, :])
                      nc.vector.scalar_tensor_tensor(
                          hla[0:64, 0:kl], yh[:, 0:kl], 0.1, yh[:, 0:kl],
                          OP.mult, OP.max)
                      nc.scalar.activation(hla[64:128, 0:kl], t2[64:128, 0:kl],
                                           AF.Gelu, bias=Ff('bv')[64:128, :],
                                           scale=Ff('gv')[64:128, :])
                      nc.sync.dma_start(ho_d.ap()[b, :, c0:c0 + kl],
                                        hla[0:64, 0:kl])
                      nc.sync.dma_start(lo_d.ap()[b, :, c0:c0 + kl],
                                        hla[64:128, 0:kl])
                      sd = statps.tile([1, 512], f32, tag="S")
                      nc.tensor.matmul(sd[:, 0:kl], Wf('wad'), hla[:, 0:kl],
                                       start=True, stop=True,
                                       skip_group_check=True,
                                       tile_position=(0, 0))
                      sdb = ssbpool.tile([1, 512], f32, tag="sdb")
                      nc.scalar.activation(sdb[:, 0:kl], sd[:, 0:kl], AF.Copy)
                      nc.sync.dma_start(sdt[q:q + 1, 0:kl], sdb[:, 0:kl])

                  # ---- batched 2-way softmax (f32 in, bf16 staged) ----
                  a0 = smaxpool.tile([GRP, 512], f32, tag="a0")
                  nc.scalar.activation(a0[:], sdt[:], AF.Sigmoid,
                                       bias=Ff('bad', GRP))
                  w0 = smaxpool.tile([GRP, 512], bf16, tag="w0")
                  nc.vector.tensor_scalar_add(w0[:], a0[:], 0.3)
                  w1_ = smaxpool.tile([GRP, 512], bf16, tag="w1_")
                  nc.scalar.activation(w1_[:], a0[:], AF.Identity,
                                       bias=Ff('c13', GRP), scale=-1.0)

                  # ---- per-pair fusion + residual + output ----
                  for q, (b, c0, kl) in gpairs:
                      hla = hla_t[q]
                      sr = sr_t[q]
                      w0s = stgpool.tile([1, 512], bf16, tag="w0s")
                      nc.sync.dma_start(w0s[:, 0:kl], w0[q:q + 1, 0:kl])
                      w1s = stgpool.tile([1, 512], bf16, tag="w1s")
                      nc.sync.dma_start(w1s[:, 0:kl], w1_[q:q + 1, 0:kl])
                      wbc = ebcps.tile([128, 512], f32, tag="ebc")
                      nc.tensor.matmul(wbc[:, 0:kl], Wf('ecol0', 1),
                                       w0s[:, 0:kl], start=True,
                                       stop=False, skip_group_check=True,
                                       tile_position=(0, 0))
                      nc.tensor.matmul(wbc[:, 0:kl], Wf('ecol1', 1),
                                       w1s[:, 0:kl], start=False,
                                       stop=True, skip_group_check=True,
                                       tile_position=(0, 0))
                      f1 = tmppool.tile([128, 512], bf16, tag="f1")
                      nc.vector.tensor_mul(f1[:, 0:kl], hla[:, 0:kl],
                                           wbc[:, 0:kl])
                      rrs = stgpool.tile([1, 512], bf16, tag="rrs")
                      nc.sync.dma_start(rrs[:, 0:kl],
                                        rstdb[3 * q + 2:3 * q + 3, 0:kl])
                      rms = stgpool.tile([1, 512], bf16, tag="rms")
                      nc.sync.dma_start(rms[:, 0:kl],
                                        mrstdb[3 * q + 2:3 * q + 3, 0:kl])
                      rr = ebcps.tile([64, 512], f32, tag="ebc")
                      nc.tensor.matmul(rr[:, 0:kl], Wf('onesr', 1),
                                       rrs[:, 0:kl],
                                       start=True, stop=True,
                                       skip_group_check=True,
                                       tile_position=(0, 0))
                      rm = ebcps.tile([64, 512], f32, tag="ebc")
                      nc.tensor.matmul(rm[:, 0:kl], Wf('onesr', 1),
                                       rms[:, 0:kl],
                                       start=True, stop=True,
                                       skip_group_check=True,
                                       tile_position=(0, 0))
                      u1 = tmppool.tile([64, 512], bf16, tag="u1")
                      nc.vector.tensor_mul(u1[:, 0:kl], sr[0:64, 0:kl],
                                           rr[:, 0:kl])
                      u2 = tmppool.tile([64, 512], bf16, tag="u2")
                      nc.vector.tensor_sub(u2[:, 0:kl], u1[:, 0:kl],
                                           rm[:, 0:kl])
                      resa = tmppool.tile([64, 512], bf16, tag="resa")
                      nc.scalar.activation(resa[:, 0:kl], u2[:, 0:kl],
                                           AF.Identity, bias=Ff('br')[0:64, :],
                                           scale=Ff('gr')[0:64, :])
                      f2 = ebcps.tile([64, 512], f32, tag="ebc")
                      nc.tensor.matmul(f2[:, 0:kl], Wf('ident2'), f1[:, 0:kl],
                                       start=True, stop=True,
                                       skip_group_check=True,
                                       tile_position=(0, 0))
                      f3 = tmppool.tile([64, 512], bf16, tag="f3")
                      nc.vector.tensor_add(f3[:, 0:kl], f2[:, 0:kl],
                                           resa[:, 0:kl])
                      nc.sync.dma_start(fo_d.ap()[b, :, c0:c0 + kl],
                                        f3[:, 0:kl])
    nc.finalize()
    return nc


# ------------------------------------------------------------------- runner
class _SpmdRunner:
    def __init__(self, nc, n_cores=NCORE):
        import jax
        from jax.sharding import Mesh, PartitionSpec
        from jax.experimental.shard_map import shard_map
        from concourse import mybir
        from concourse.bass2jax import (_bass_exec_p, install_neuronx_cc_hook,
                                        partition_id_tensor)
        install_neuronx_cc_hook()
        self.jax = jax
        self.n_cores = n_cores
        partition_name = (nc.partition_id_tensor.name
                          if nc.partition_id_tensor else None)
        in_names, out_names, out_avals, zero_outs = [], [], [], []
        for alloc in nc.m.functions[0].allocations:
            if not isinstance(alloc, mybir.MemoryLocationSet):
                continue
            name = alloc.memorylocations[0].name
            if alloc.kind == "ExternalInput":
                if name != partition_name:
                    in_names.append(name)
            elif alloc.kind == "ExternalOutput":
                out_names.append(name)
                shape = tuple(alloc.tensor_shape)
                dtype = mybir.dt.np(alloc.dtype)
                out_avals.append(jax.core.ShapedArray(shape, dtype))
                zero_outs.append(np.zeros(shape, dtype))
        self.in_names, self.out_names = in_names, out_names
        self.out_avals = out_avals
        n_params, n_outs = len(in_names), len(out_avals)
        all_in = list(in_names) + list(out_names)
        if partition_name is not None:
            all_in.append(partition_name)

        def _body(*args):
            operands = list(args)
            if partition_name is not None:
                operands.append(partition_id_tensor())
            outs = _bass_exec_p.bind(
                *operands, out_avals=tuple(out_avals),
                in_names=tuple(all_in), out_names=tuple(out_names),
                lowering_input_output_aliases=(),
                sim_require_finite=True, sim_require_nnan=True, nc=nc)
            return tuple(outs)

        devices = jax.devices()[:n_cores]
        mesh = Mesh(np.asarray(devices), ("core",))
        self.mesh = mesh
        in_specs = (PartitionSpec("core"),) * (n_params + n_outs)
        out_specs = (PartitionSpec("core"),) * n_outs
        self.fn = jax.jit(
            shard_map(_body, mesh=mesh, in_specs=in_specs,
                      out_specs=out_specs, check_rep=False),
            keep_unused=True)
        self._concat_zeros = [
            np.zeros((n_cores * z.shape[0], *z.shape[1:]), z.dtype)
            for z in zero_outs]

    def prepare(self, in_maps):
        from jax.sharding import NamedSharding, PartitionSpec
        n = self.n_cores
        per_core = [[np.ascontiguousarray(m[name]) for name in self.in_names]
                    for m in in_maps]
        concat_in = [np.concatenate([per_core[c][i] for c in range(n)], axis=0)
                     for i in range(len(self.in_names))]
        args = concat_in + self._concat_zeros
        # Pre-shard along dim 0 so each run() call dispatches the kernel
        # directly instead of inserting per-call resharding copies.
        sh = NamedSharding(self.mesh, PartitionSpec("core"))
        return [self.jax.device_put(a, sh) for a in args]

    def run(self, args):
        outs = self.fn(*args)
        self.jax.block_until_ready(outs)
        return outs

    def split_outs(self, outs):
        res = []
        for c in range(self.n_cores):
            d = {}
            for i, name in enumerate(self.out_names):
                d[name] = np.asarray(outs[i]).reshape(
                    self.n_cores, *self.out_avals[i].shape)[c]
            res.append(d)
        return res


# -------------------------------------------------------------------- entry
def _get(inputs):
    gb = _prep_graph(inputs['edge_index'])
    gc = _prep_graph(inputs['causal_edge_index'])
    b_pad = max(128, -(-int(max(gb['counts'].max(), gc['counts'].max()))
                     // 128) * 128)
    stream = NBLK * b_pad
    nseg = stream // 128
    nch = -(-stream // 1024)
    cwb, cwf, ba_diff = _fold_weights(inputs)
    key = (b_pad, nseg, nch, round(ba_diff, 9), REPEAT)
    if key not in _cache:
        nc = _build_program(nseg, nch, ba_diff, REPEAT)
        _cache[key] = _SpmdRunner(nc)
    return _cache[key], gb, gc, b_pad, nseg, nch, cwb, cwf


def make_in_maps(inputs):
    runner, gb, gc, b_pad, nseg, nch, cwb, cwf = _get(inputs)
    x = np.asarray(inputs['x'], np.float32)
    xflat = np.zeros((128, N), np.float32)
    xflat[0:96] = x.reshape(96, N)
    xflat[96] = 1.0
    xT = np.zeros((NROWS, 128), np.float32)
    xT[:N, 0:97] = xflat[0:97].T
    xT16 = xT.astype(bfloat16)
    idx_b, oh_b = _build_streams(gb, b_pad, nseg, nch)
    idx_c, oh_c = _build_streams(gc, b_pad, nseg, nch)
    in_maps = []
    for c in range(NCORE):
        xs = np.zeros((128, NPCP), np.float32)
        xs[:, 0:NPC] = xflat[:, c * NPC:(c + 1) * NPC]
        in_maps.append({
            'xT': xT16, 'xsb': xs.astype(bfloat16), 'cwb': cwb, 'cwf': cwf,
            'rep_tag': np.zeros((1, 64 * REPEAT), np.float32),
            'idx_b': idx_b[c], 'oh_b': oh_b[c],
            'idx_c': idx_c[c], 'oh_c': oh_c[c],
        })
    return runner, in_maps


def kernel(**inputs):
    runner, in_maps = make_in_maps(inputs)
    args = runner.prepare(in_maps)
    outs = runner.run(args)
    res = runner.split_outs(outs)
    fused = np.empty((B, OD, N), np.float32)
    high = np.empty((B, OD, N), np.float32)
    low = np.empty((B, OD, N), np.float32)
    for c in range(NCORE):
        sl = slice(c * NPC, (c + 1) * NPC)
        fused[:, :, sl] = res[c]['fo'][:, :, 0:NPC].astype(np.float32)
        high[:, :, sl] = res[c]['ho'][:, :, 0:NPC].astype(np.float32)
        low[:, :, sl] = res[c]['lo'][:, :, 0:NPC].astype(np.float32)
    return fused, high, low
